# revision 18
# baseline (speedup 1.0000x reference)
"""DenoiseNet loss kernel for 8 Trainium2 NeuronCores.

Strategy: pure data parallel over the batch (4 batches/core). PointNet MLP in
fp16 (fp32 PSUM accumulate), exact global BatchNorm via per-layer AllReduce of
(sum, sumsq). KNN argmin via hi/lo-fp16-split matmul of 2q.r-|r|^2, extraction
of the matched |v-r*|^2 by is_equal one-hot + fused multiply-reduce.

v2 restructure for PE warmth / engine balance:
 - weights host-packed per layer, one DMA per (iter, layer), prefetched at
   iteration start.
 - PSUM evictions on the vector engine with fused sum-accumulate; sumsq
   pass split between scalar/vector; BN affine+ReLU fused into a single
   scalar-engine activation per 128-channel chunk (true per-channel scale,
   no weight folding).
 - KNN matmuls of phase1(iter i) and phase4(iter i-1) are interleaved into
   the 7 BN-AllReduce stall windows of iter i, keeping the PE busy (HAM
   stays at full clock) while the collective is in flight.
"""
import numpy as np

import concourse.bass as bass
import concourse.mybir as mybir
import concourse.tile as tile
from concourse import bacc
from concourse.bass_utils import run_bass_kernel_spmd

dt = mybir.dt
F32 = dt.float32
F16 = dt.float16
AF = mybir.ActivationFunctionType
OP = mybir.AluOpType
AX = mybir.AxisListType

B, N, NCORES = 32, 1000, 8
BL = B // NCORES            # 4 batches per core
PTS = BL * N                # 4000 points per core
NITER = 4
NPTS_GLOBAL = B * N         # 32000 (BN population)
EPS = 1e-5
NOISE_DECAY = 4.0
QT = 125                    # q tile (8 per batch)
RP = 1024                   # padded ref points (24 sentinels)
PT = 500                    # pts tile (8 tiles of 500)
NPT = PTS // PT             # 8
NG = NPT // 2               # 4 psum groups of 2 tiles

# (C_in, C_out, has_bn) per layer
LAYERS = [(3, 64, 1), (64, 128, 1), (128, 256, 1), (256, 512, 1),
          (512, 1024, 1), (1024, 512, 1), (512, 256, 1), (256, 3, 0)]
NCI = [max(1, ci // 128) for ci, co, _ in LAYERS]
NCO = [max(1, (co + 127) // 128) for ci, co, _ in LAYERS]
# gh packing offsets for the 7 BN layers
GHOFF = [0]
for _l in range(7):
    GHOFF.append(GHOFF[-1] + NCO[_l])
GHTOT = GHOFF[-1]           # 22

# zt buffer assignment: even layers write pool A (8 bufs), odd write pool B (4)
ZPOOL = ['A', 'B', 'A', 'B', 'A', 'B', 'A', None]

# KNN fill distribution: tiles emitted at each of the 7 BN boundaries
P1_SPLIT = [5, 5, 5, 5, 4, 4, 4]
P4_SPLIT = [5, 5, 5, 5, 4, 4, 4]

RG = [list(range(NCORES))]

_NC_CACHE = {}


def _build(niter=NITER, nlayers=8, do_knn=True, do_ar=True):
    nc = bacc.Bacc(None, target_bir_lowering=False, debug=False)

    x0t_d = nc.dram_tensor("x0t", [3, PTS], F32, kind="ExternalInput")
    cneg_d = nc.dram_tensor("cneg", [2, PTS], F16, kind="ExternalInput")
    cpos_d = nc.dram_tensor("cpos", [2, PTS], F16, kind="ExternalInput")
    sw_d = nc.dram_tensor("sw", [128, 32], F32, kind="ExternalInput")
    sws3_d = nc.dram_tensor("sws3", [3, PTS], F16, kind="ExternalInput")
    db3_d = nc.dram_tensor("db3t", [3, NITER], F32, kind="ExternalInput")
    r_d = [[nc.dram_tensor(f"rknn_{i}_{b}", [13, RP], F16, kind="ExternalInput")
            for b in range(BL)] for i in range(NITER)]
    w_d = [[nc.dram_tensor(
        f"w_{i}_{l}",
        [LAYERS[l][0], LAYERS[l][1]] if LAYERS[l][0] <= 128
        else [128, NCI[l] * LAYERS[l][1]], F16, kind="ExternalInput")
        for l in range(8)] for i in range(NITER)]
    gh_d = [nc.dram_tensor(f"gh_{i}", [128, 2, GHTOT], F32,
                           kind="ExternalInput") for i in range(NITER)]
    loss_d = nc.dram_tensor("loss_part", [128, 1], F32, kind="ExternalOutput")
    loss3_d = nc.dram_tensor("loss_part3", [3, 1], F32, kind="ExternalOutput")

    with tile.TileContext(nc) as tc:
        with (
            tc.tile_pool(name="sb", bufs=1) as sb,
            tc.tile_pool(name="ps", bufs=2, space="PSUM") as ps,
            tc.tile_pool(name="psk", bufs=2, space="PSUM") as psk,
            tc.tile_pool(name="dram", bufs=2, space="DRAM") as dram,
        ):
            # ---------- persistent setup ----------
            sw_sb = sb.tile([128, 32], F32, tag="sw")
            nc.gpsimd.dma_start(sw_sb[:], sw_d[:])
            sws3 = sb.tile([3, PTS], F16, tag="sws3")
            nc.gpsimd.dma_start(sws3[:], sws3_d[:])
            db3_sb = sb.tile([3, NITER], F32, tag="db3")
            nc.gpsimd.dma_start(db3_sb[:], db3_d[:])

            x_cur = sb.tile([3, PTS], F32, tag="xA")
            nc.gpsimd.dma_start(x_cur[:], x0t_d[:])

            eps_sb = sb.tile([128, 1], F32, tag="epsc")
            nc.vector.memset(eps_sb[:], float(EPS))
            licol = sb.tile([128, NITER], F32, tag="licol")
            nc.vector.memset(licol[:], 0.0)
            a3col = sb.tile([3, NITER], F32, tag="a3col")
            nc.vector.memset(a3col[:], 0.0)

            # zt activation pools (3D tiles [128, NPT, PT])
            zA = [sb.tile([128, NPT, PT], F16, tag=f"zA{co}", name=f"zA{co}")
                  for co in range(8)]
            zB = [sb.tile([128, NPT, PT], F16, tag=f"zB{co}", name=f"zB{co}")
                  for co in range(4)]
            junk16 = sb.tile([128, NPT, PT], F16, tag="junk16")
            scr3 = sb.tile([3, PTS], F16, tag="scr3")
            # aliases: fp16 x for layer0 lives in junk16's first rows (safe:
            # the first junk write of an iter comes after L0's matmuls);
            # tanh preds live in zB[0]'s first rows (zB is free at L7 time).
            xf16 = junk16
            preds16 = zB[0]

            # per-iteration state (rotating buffers across iterations)
            state = {}

            def it_state(it):
                if it in state:
                    return state[it]
                s = {
                    'Ld': sb.tile([11, PTS], F16, tag="Ld", bufs=2,
                                  name=f"Ld_{it}"),
                    'Le': sb.tile([11, PTS], F16, tag="Le", bufs=1,
                                  name=f"Le_{it}"),
                    'r': [sb.tile([13, RP], F16, tag=f"r{b}", bufs=2,
                                  name=f"r_{it}_{b}") for b in range(BL)],
                    'm': sb.tile([128, 32], F32, tag="m_all", bufs=2,
                                 name=f"m_{it}"),
                    'S': sb.tile([128, 32], F32, tag="S", bufs=2,
                                 name=f"S_{it}"),
                }
                state[it] = s
                return s

            def emit_p1(it, t):
                """KNN phase1 tile: d-matmuls + row max for (b, qt)."""
                s = it_state(it)
                b, qt = t // 8, t % 8
                col = t
                qsl = slice(b * N + qt * QT, b * N + (qt + 1) * QT)
                kpd = psk.tile([QT, 2, 512], F32, tag="kpd")
                for rt in range(2):
                    nc.tensor.matmul(kpd[:, rt, :], s['Ld'][0:11, qsl],
                                     s['r'][b][0:11, rt * 512:(rt + 1) * 512],
                                     start=True, stop=True)
                nc.vector.tensor_reduce(s['m'][0:QT, col:col + 1], kpd[:],
                                        AX.XY, OP.max)

            def emit_p1j(it, t):
                """Junk KNN d-matmuls (no consumer): pure PE boundary fill.
                Chained behind the gated real tiles via the psk pool, so it
                executes in the AllReduce/affine tail windows."""
                s = it_state(it)
                b, qt = t // 8, t % 8
                qsl = slice(b * N + qt * QT, b * N + (qt + 1) * QT)
                kpd = psk.tile([QT, 2, 512], F32, tag="kpd", name="kpdj")
                for rt in range(2):
                    nc.tensor.matmul(kpd[:, rt, :], s['Ld'][0:11, qsl],
                                     s['r'][b][0:11, rt * 512:(rt + 1) * 512],
                                     start=True, stop=True)

            def emit_p4(it, t):
                """KNN phase4 tile: recompute d, compute e, extract."""
                s = it_state(it)
                b, qt = t // 8, t % 8
                col = t
                qsl = slice(b * N + qt * QT, b * N + (qt + 1) * QT)
                kpd = psk.tile([QT, 2, 512], F32, tag="kpd")
                for rt in range(2):
                    nc.tensor.matmul(kpd[:, rt, :], s['Ld'][0:11, qsl],
                                     s['r'][b][0:11, rt * 512:(rt + 1) * 512],
                                     start=True, stop=True)
                kpe = psk.tile([QT, 2, 512], F32, tag="kpd")
                for rt in range(2):
                    nc.tensor.matmul(kpe[:, rt, :], s['Le'][0:11, qsl],
                                     s['r'][b][0:11, rt * 512:(rt + 1) * 512],
                                     start=True, stop=True)
                o16 = sb.tile([QT, 2, 512], F16, tag="o16", bufs=2)
                nc.vector.tensor_scalar(o16[:], kpd[:],
                                        s['m'][0:QT, col:col + 1], None,
                                        OP.is_equal)
                nc.vector.scalar_tensor_tensor(
                    o16[:], kpe[:], 1.0, o16[:], OP.mult, OP.mult,
                    accum_out=s['S'][0:QT, col:col + 1])

            # ---------- main loop ----------
            for it in range(niter):
                s = it_state(it)

                # prefetch: weights, gh, r tiles for this iteration
                wts = []
                for l in range(nlayers):
                    cin, cout = LAYERS[l][:2]
                    wt = sb.tile(
                        [cin, cout] if cin <= 128 else [128, NCI[l] * cout],
                        F16, tag=f"w{l}", name=f"w_{it}_{l}")
                    nc.gpsimd.dma_start(wt[:], w_d[it][l][:])
                    wts.append(wt)
                gh = sb.tile([128, 2, GHTOT], F32, tag="gh", bufs=2)
                nc.gpsimd.dma_start(gh[:], gh_d[it][:])
                if do_knn:
                    for b in range(BL):
                        nc.gpsimd.dma_start(s['r'][b][:], r_d[it][b][:])
                    nc.vector.memset(s['S'][:], 0.0)

                # fp16 copy of x for layer 0 (into junk16 alias rows) --
                # first, so L0's matmuls are not gated on the Ld/Le builds
                nc.vector.tensor_copy(xf16[0:3, :, :], x_cur[:])

                p1c = [0]
                p4c = [0]
                pj = [0]

                if do_knn:
                    # build Ld(it) = [2x_h, 2x_l, 2x_h, -1, -1]
                    nc.vector.tensor_scalar_mul(s['Ld'][0:3, :], x_cur[:], 2.0)
                    nc.vector.scalar_tensor_tensor(
                        scr3[:], x_cur[:], 2.0, s['Ld'][0:3, :], OP.mult,
                        OP.subtract)
                    nc.gpsimd.dma_start(s['Ld'][3:6, :], scr3[:])
                    nc.gpsimd.dma_start(s['Ld'][6:9, :], s['Ld'][0:3, :])
                    nc.gpsimd.dma_start(s['Ld'][9:11, :], cneg_d[:])

                    if it > 0:
                        # build Le(it-1) from x_cur (post-update coords of
                        # iter it-1), plus term2 for iter it-1
                        sp = it_state(it - 1)
                        nc.vector.tensor_scalar_mul(sp['Le'][0:3, :],
                                                    x_cur[:], -2.0)
                        nc.vector.scalar_tensor_tensor(
                            scr3[:], x_cur[:], -2.0, sp['Le'][0:3, :],
                            OP.mult, OP.subtract)
                        nc.gpsimd.dma_start(sp['Le'][3:6, :], scr3[:])
                        nc.gpsimd.dma_start(sp['Le'][6:9, :], sp['Le'][0:3, :])
                        nc.gpsimd.dma_start(sp['Le'][9:11, :], cpos_d[:])
                        # term2: sum_q sw_q |x|^2 via (x*sqrt(sw))^2
                        nc.vector.tensor_tensor(out=scr3[:], in0=x_cur[:],
                                                in1=sws3[:], op=OP.mult)
                        nc.vector.scalar_tensor_tensor(
                            preds16[0:3, :, :], scr3[:], 1.0, scr3[:],
                            OP.mult, OP.mult,
                            accum_out=a3col[0:3, it - 1:it])

                def gate_col(m_tile, col, gate_ap):
                    # value-preserving write to m[col] that reads gate_ap:
                    # m[col] = 0*gate + m[col].  Pins the KNN batch to the
                    # boundary (scheduler cannot hoist it before the gate).
                    nc.vector.scalar_tensor_tensor(
                        m_tile[0:QT, col:col + 1], gate_ap, 0.0,
                        m_tile[0:QT, col:col + 1], OP.mult, OP.add)

                def knn_batch(bi, gate_ap=None):
                    """Emit the KNN fill slice for boundary bi of iter it."""
                    if not do_knn:
                        return
                    if gate_ap is not None and p1c[0] < 32:
                        gate_col(s['m'], p1c[0], gate_ap)
                    for _ in range(P1_SPLIT[bi]):
                        if p1c[0] < 32:
                            emit_p1(it, p1c[0])
                            p1c[0] += 1
                    if it > 0:
                        if gate_ap is not None and p4c[0] < 32:
                            gate_col(it_state(it - 1)['m'], p4c[0], gate_ap)
                        for _ in range(P4_SPLIT[bi]):
                            if p4c[0] < 32:
                                emit_p4(it - 1, p4c[0])
                                p4c[0] += 1
                    nj = 9 if it == 0 else 5
                    for j in range(nj):
                        emit_p1j(it, (pj[0] + j) % 32)
                    pj[0] += nj

                # ---------- MLP ----------
                zin = None
                bi = 0
                for l in range(nlayers):
                    cin, cout, has_bn = LAYERS[l]
                    nci, nco = NCI[l], NCO[l]
                    CIP = min(128, cin)
                    zout = zA if ZPOOL[l] == 'A' else zB
                    wt = wts[l]

                    if has_bn:
                        sums = sb.tile([128, nco, NG], F32, tag="sums", bufs=2)
                        ssq = sb.tile([128, nco, NG], F32, tag="ssq", bufs=2)
                        statsr = sb.tile([128, 2, nco], F32, tag="statsr",
                                         bufs=2)

                    for g in range(NG):
                        for co in range(nco):
                            CO = min(128, cout - co * 128)
                            zp = ps.tile([128, 2, 512], F32, tag="zp")
                            for ci in range(nci):
                                if l == 0:
                                    lhs = wt[0:CIP, co * 128:co * 128 + CO]
                                else:
                                    lhs = wt[0:CIP,
                                             ci * cout + co * 128:
                                             ci * cout + co * 128 + CO]
                                for h in range(2):
                                    pt = 2 * g + h
                                    if l == 0:
                                        rhs = xf16[0:3, pt, :]
                                    else:
                                        rhs = zin[ci][0:CIP, pt, :]
                                    nc.tensor.matmul(
                                        zp[0:CO, h, 0:PT], lhs, rhs,
                                        start=(ci == 0), stop=(ci == nci - 1))
                            if has_bn:
                                nc.vector.tensor_scalar(
                                    zout[co][0:CO, 2 * g:2 * g + 2, :],
                                    zp[0:CO, :, 0:PT], 1.0, 0.0, OP.mult,
                                    OP.add,
                                    accum_out=sums[0:CO, co, g:g + 1])
                                # sumsq for this (co, g) slice
                                if co % 2 == 0:
                                    nc.scalar.activation(
                                        junk16[0:CO, 2 * g:2 * g + 2, :],
                                        zout[co][0:CO, 2 * g:2 * g + 2, :],
                                        AF.Square,
                                        accum_out=ssq[0:CO, co, g:g + 1])
                                else:
                                    nc.vector.scalar_tensor_tensor(
                                        junk16[0:CO, 2 * g:2 * g + 2, :],
                                        zout[co][0:CO, 2 * g:2 * g + 2, :],
                                        1.0,
                                        zout[co][0:CO, 2 * g:2 * g + 2, :],
                                        OP.mult, OP.mult,
                                        accum_out=ssq[0:CO, co, g:g + 1])
                            else:
                                nc.scalar.activation(
                                    preds16[0:3, 2 * g:2 * g + 2, :],
                                    zp[0:3, :, 0:PT], AF.Tanh,
                                    bias=db3_sb[:, it:it + 1])

                    if not has_bn:
                        break

                    nc.vector.tensor_reduce(statsr[:, 0, :], sums[:], AX.X,
                                            OP.add)
                    nc.vector.tensor_reduce(statsr[:, 1, :], ssq[:], AX.X,
                                            OP.add)
                    arin = dram.tile([128, 2, nco], F32, tag="arin")
                    arout = dram.tile([128, 2, nco], F32, tag="arout")
                    nc.gpsimd.dma_start(arin[:], statsr[:])
                    if do_ar:
                        nc.gpsimd.collective_compute(
                            "AllReduce", OP.add, replica_groups=RG,
                            ins=[arin.opt()], outs=[arout.opt()])
                    else:
                        nc.gpsimd.dma_start(arout[:], arin[:])

                    # KNN fill while the AllReduce is in flight (gated on
                    # this layer's stats so the scheduler can't hoist it)
                    knn_batch(bi, statsr[0:QT, 0, 0:1])
                    bi += 1

                    statsg = sb.tile([128, 2, nco], F32, tag="statsg", bufs=2)
                    nc.gpsimd.dma_start(statsg[:], arout[:])

                    # affine coefs: a = g/sqrt(var+eps); ct = h - mean*a
                    af = sb.tile([128, 6, nco], F32, tag="af", bufs=2)
                    inv_n = 1.0 / NPTS_GLOBAL
                    nc.vector.tensor_scalar_mul(af[:, 0:2, :], statsg[:],
                                                inv_n)
                    nc.vector.tensor_tensor(out=af[:, 2, :], in0=af[:, 0, :],
                                            in1=af[:, 0, :], op=OP.mult)
                    nc.vector.tensor_tensor(out=af[:, 1, :], in0=af[:, 1, :],
                                            in1=af[:, 2, :], op=OP.subtract)
                    nc.scalar.activation(af[:, 2, :], af[:, 1, :], AF.Sqrt,
                                         bias=eps_sb[:])
                    nc.vector.reciprocal(af[:, 3, :], af[:, 2, :])
                    nc.vector.tensor_tensor(
                        out=af[:, 4, :], in0=gh[:, 0, GHOFF[l]:GHOFF[l] + nco],
                        in1=af[:, 3, :], op=OP.mult)
                    nc.vector.tensor_tensor(out=af[:, 2, :], in0=af[:, 0, :],
                                            in1=af[:, 4, :], op=OP.mult)
                    nc.vector.tensor_tensor(
                        out=af[:, 5, :], in0=gh[:, 1, GHOFF[l]:GHOFF[l] + nco],
                        in1=af[:, 2, :], op=OP.subtract)

                    # affine+relu at (chunk, group) granularity, g-major
                    # so the next layer's first psum chains unblock early;
                    # chunks alternate scalar/vector engines
                    for g in range(NG):
                        for co in range(nco):
                            CO = min(128, cout - co * 128)
                            dst = zout[co][0:CO, 2 * g:2 * g + 2, :]
                            sc = af[0:CO, 4, co:co + 1]
                            bi_ = af[0:CO, 5, co:co + 1]
                            if co % 2 == 0:
                                nc.scalar.activation(dst, dst, AF.Relu,
                                                     bias=bi_, scale=sc)
                            else:
                                nc.vector.tensor_scalar(
                                    dst, dst, sc, bi_, OP.mult, OP.add)
                                nc.vector.tensor_scalar_max(dst, dst, 0.0)
                    zin = zout

                # ---------- x update (in place) ----------
                if nlayers < 8:
                    continue
                nc.vector.tensor_tensor(out=x_cur[:], in0=x_cur[:],
                                        in1=preds16[0:3, :, :], op=OP.add)

                if do_knn and it < niter - 1:
                    # iteration-transition fill (L7/prologue stretch)
                    for j in range(4):
                        emit_p1j(it, (pj[0] + j) % 32)
                    pj[0] += 4

                if do_knn and it == niter - 1:
                    # tail: build Le(last)+term2(last), then all of phase4
                    nc.vector.tensor_scalar_mul(s['Le'][0:3, :], x_cur[:],
                                                -2.0)
                    nc.vector.scalar_tensor_tensor(
                        scr3[:], x_cur[:], -2.0, s['Le'][0:3, :], OP.mult,
                        OP.subtract)
                    nc.gpsimd.dma_start(s['Le'][3:6, :], scr3[:])
                    nc.gpsimd.dma_start(s['Le'][6:9, :], s['Le'][0:3, :])
                    nc.gpsimd.dma_start(s['Le'][9:11, :], cpos_d[:])
                    nc.vector.tensor_tensor(out=scr3[:], in0=x_cur[:],
                                            in1=sws3[:], op=OP.mult)
                    nc.vector.scalar_tensor_tensor(
                        preds16[0:3, :, :], scr3[:], 1.0, scr3[:], OP.mult,
                        OP.mult, accum_out=a3col[0:3, it:it + 1])
                    for t in range(32):
                        emit_p4(it, t)

                # loss column for iter it-1 (S complete after its 32 p4 tiles)
                if do_knn and it > 0:
                    sp = it_state(it - 1)
                    jk2 = sb.tile([128, 32], F32, tag="jk2", bufs=2)
                    nc.vector.scalar_tensor_tensor(
                        jk2[:], sp['S'][:], 1.0, sw_sb[:], OP.mult, OP.mult,
                        accum_out=licol[:, it - 1:it])

            if do_knn and niter >= 1 and nlayers >= 8:
                sl = it_state(niter - 1)
                jk2 = sb.tile([128, 32], F32, tag="jk2", bufs=2)
                nc.vector.scalar_tensor_tensor(
                    jk2[:], sl['S'][:], 1.0, sw_sb[:], OP.mult, OP.mult,
                    accum_out=licol[:, niter - 1:niter])

            lacc = sb.tile([128, 1], F32, tag="laccA")
            nc.vector.tensor_reduce(lacc[:], licol[:], AX.X, OP.add)
            acc3f = sb.tile([3, 1], F32, tag="acc3A")
            nc.vector.tensor_reduce(acc3f[:], a3col[0:3, :], AX.X, OP.add)
            nc.gpsimd.dma_start(loss_d[:], lacc[:])
            nc.gpsimd.dma_start(loss3_d[:], acc3f[:])
    nc.compile()
    return nc


def _host_prep(inputs):
    """Build per-core input maps."""
    f32 = np.float32
    noisy = np.asarray(inputs["pcl_noisy"], f32)
    clean = np.asarray(inputs["pcl_clean"], f32)
    seeds = np.asarray(inputs["pcl_seeds"], f32)
    std = np.asarray(inputs["pcl_std"], f32)
    noise = np.asarray(inputs["noise"], f32)

    pn = noisy - seeds
    pc = clean - seeds
    sdist = np.sum(pn.astype(np.float64) ** 2, -1, keepdims=True)
    max_sq = sdist[:, -1:, :]
    sw = np.exp(-sdist * 9.0 / max_sq)[..., 0]
    sw = (sw / sw.sum(1, keepdims=True))  # [B, N] float64

    tgts = []
    cur = std.copy()
    for i in range(NITER):
        if i < NITER - 1:
            cur = cur / NOISE_DECAY
            tgts.append(pc + noise[i] * cur[:, None, None])
        else:
            tgts.append(pc.copy())

    sent = np.full((RP - N, 3), 100.0, np.float64)

    shared = {}
    for i in range(NITER):
        for l in range(8):
            key = f'ew{l+1}' if l < 5 else f'dw{l-4}'
            w = np.asarray(inputs[key], f32)[i].astype(np.float16)
            cin, cout = LAYERS[l][:2]
            if cin <= 128:
                shared[f"w_{i}_{l}"] = w
            else:
                nci = NCI[l]
                shared[f"w_{i}_{l}"] = np.ascontiguousarray(
                    w.reshape(nci, 128, cout).transpose(1, 0, 2).reshape(
                        128, nci * cout))
        gharr = np.zeros((128, 2, GHTOT), f32)
        for l in range(7):
            nco = NCO[l]
            cout = LAYERS[l][1]
            gk = f'eg{l+1}' if l < 5 else f'dg{l-4}'
            hk = f'eh{l+1}' if l < 5 else f'dh{l-4}'
            g = np.asarray(inputs[gk], f32)[i]
            h = np.asarray(inputs[hk], f32)[i]
            gp = np.zeros(nco * 128, f32); gp[:cout] = g
            hp = np.zeros(nco * 128, f32); hp[:cout] = h
            gharr[:, 0, GHOFF[l]:GHOFF[l] + nco] = gp.reshape(nco, 128).T
            gharr[:, 1, GHOFF[l]:GHOFF[l] + nco] = hp.reshape(nco, 128).T
        shared[f"gh_{i}"] = gharr
    shared["db3t"] = np.ascontiguousarray(np.asarray(inputs["db3"], f32).T)
    shared["cneg"] = np.full((2, PTS), -1.0, np.float16)
    shared["cpos"] = np.full((2, PTS), 1.0, np.float16)

    in_maps = []
    for c in range(NCORES):
        bs = slice(c * BL, (c + 1) * BL)
        m = dict(shared)
        m["x0t"] = np.ascontiguousarray(
            pn[bs].transpose(2, 0, 1).reshape(3, PTS))
        swc = np.zeros((128, 32), f32)
        for b in range(BL):
            for qt in range(8):
                swc[0:QT, b * 8 + qt] = sw[c * BL + b,
                                           qt * QT:(qt + 1) * QT].astype(f32)
        m["sw"] = swc
        m["sws3"] = np.broadcast_to(
            np.sqrt(sw[bs]).reshape(1, PTS), (3, PTS)).astype(np.float16)
        for i in range(NITER):
            for b in range(BL):
                coords = np.concatenate(
                    [tgts[i][c * BL + b].astype(np.float64), sent], 0)
                rh = coords.astype(np.float16)
                rl = (coords - rh.astype(np.float64)).astype(np.float16)
                rsq = (coords ** 2).sum(1)
                rsqh = rsq.astype(np.float16)
                rsql = (rsq - rsqh.astype(np.float64)).astype(np.float16)
                R = np.empty((13, RP), np.float16)
                R[0:3] = rh.T
                R[3:6] = rh.T
                R[6:9] = rl.T
                R[9] = rsqh
                R[10] = rsql
                R[11] = 1.0
                R[12] = 1.0
                m[f"rknn_{i}_{b}"] = R
        in_maps.append(m)
    return in_maps


def kernel(**inputs):
    if "nc" not in _NC_CACHE:
        _NC_CACHE["nc"] = _build()
    nc = _NC_CACHE["nc"]
    in_maps = _host_prep(inputs)
    res = run_bass_kernel_spmd(nc, in_maps, list(range(NCORES))).results
    total = 0.0
    for c in range(NCORES):
        total += float(res[c]["loss_part"].sum())
        total += float(res[c]["loss_part3"].sum())
    return np.asarray(total / B, dtype=np.float32)


# revision 19
# speedup vs baseline: 1.0397x; 1.0397x over previous
"""DenoiseNet loss kernel for 8 Trainium2 NeuronCores.

Strategy: pure data parallel over the batch (4 batches/core). PointNet MLP in
fp16 (fp32 PSUM accumulate), exact global BatchNorm via per-layer AllReduce of
(sum, sumsq). KNN argmin via hi/lo-fp16-split matmul of 2q.r-|r|^2, extraction
of the matched |v-r*|^2 by is_equal one-hot + fused multiply-reduce.

v2 restructure for PE warmth / engine balance:
 - weights host-packed per layer, one DMA per (iter, layer), prefetched at
   iteration start.
 - PSUM evictions on the vector engine with fused sum-accumulate; sumsq
   pass split between scalar/vector; BN affine+ReLU fused into a single
   scalar-engine activation per 128-channel chunk (true per-channel scale,
   no weight folding).
 - KNN matmuls of phase1(iter i) and phase4(iter i-1) are interleaved into
   the 7 BN-AllReduce stall windows of iter i, keeping the PE busy (HAM
   stays at full clock) while the collective is in flight.
"""
import numpy as np

import concourse.bass as bass
import concourse.mybir as mybir
import concourse.tile as tile
from concourse import bacc
from concourse.bass_utils import run_bass_kernel_spmd

dt = mybir.dt
F32 = dt.float32
F16 = dt.float16
AF = mybir.ActivationFunctionType
OP = mybir.AluOpType
AX = mybir.AxisListType

B, N, NCORES = 32, 1000, 8
BL = B // NCORES            # 4 batches per core
PTS = BL * N                # 4000 points per core
NITER = 4
NPTS_GLOBAL = B * N         # 32000 (BN population)
EPS = 1e-5
NOISE_DECAY = 4.0
QT = 125                    # q tile (8 per batch)
RP = 1024                   # padded ref points (24 sentinels)
PT = 500                    # pts tile (8 tiles of 500)
NPT = PTS // PT             # 8
NG = NPT // 2               # 4 psum groups of 2 tiles

# (C_in, C_out, has_bn) per layer
LAYERS = [(3, 64, 1), (64, 128, 1), (128, 256, 1), (256, 512, 1),
          (512, 1024, 1), (1024, 512, 1), (512, 256, 1), (256, 3, 0)]
NCI = [max(1, ci // 128) for ci, co, _ in LAYERS]
NCO = [max(1, (co + 127) // 128) for ci, co, _ in LAYERS]
# gh packing offsets for the 7 BN layers
GHOFF = [0]
for _l in range(7):
    GHOFF.append(GHOFF[-1] + NCO[_l])
GHTOT = GHOFF[-1]           # 22

# zt buffer assignment: even layers write pool A (8 bufs), odd write pool B (4)
ZPOOL = ['A', 'B', 'A', 'B', 'A', 'B', 'A', None]

# KNN fill distribution: tiles emitted at each of the 7 BN boundaries
P1_SPLIT = [5, 5, 5, 5, 4, 4, 4]
P4_SPLIT = [5, 5, 5, 5, 4, 4, 4]

RG = [list(range(NCORES))]

_NC_CACHE = {}


def _build(niter=NITER, nlayers=8, do_knn=True, do_ar=True):
    nc = bacc.Bacc(None, target_bir_lowering=False, debug=False)

    x0t_d = nc.dram_tensor("x0t", [3, PTS], F32, kind="ExternalInput")
    cneg_d = nc.dram_tensor("cneg", [2, PTS], F16, kind="ExternalInput")
    cpos_d = nc.dram_tensor("cpos", [2, PTS], F16, kind="ExternalInput")
    sw_d = nc.dram_tensor("sw", [128, 32], F32, kind="ExternalInput")
    sws3_d = nc.dram_tensor("sws3", [3, PTS], F16, kind="ExternalInput")
    db3_d = nc.dram_tensor("db3t", [3, NITER], F32, kind="ExternalInput")
    r_d = [[nc.dram_tensor(f"rknn_{i}_{b}", [13, RP], F16, kind="ExternalInput")
            for b in range(BL)] for i in range(NITER)]
    w_d = [[nc.dram_tensor(
        f"w_{i}_{l}",
        [LAYERS[l][0], LAYERS[l][1]] if LAYERS[l][0] <= 128
        else [128, NCI[l] * LAYERS[l][1]], F16, kind="ExternalInput")
        for l in range(8)] for i in range(NITER)]
    gh_d = [nc.dram_tensor(f"gh_{i}", [128, 2, GHTOT], F32,
                           kind="ExternalInput") for i in range(NITER)]
    loss_d = nc.dram_tensor("loss_part", [128, 1], F32, kind="ExternalOutput")
    loss3_d = nc.dram_tensor("loss_part3", [3, 1], F32, kind="ExternalOutput")

    with tile.TileContext(nc) as tc:
        with (
            tc.tile_pool(name="sb", bufs=1) as sb,
            tc.tile_pool(name="ps", bufs=2, space="PSUM") as ps,
            tc.tile_pool(name="psk", bufs=2, space="PSUM") as psk,
            tc.tile_pool(name="dram", bufs=2, space="DRAM") as dram,
        ):
            # ---------- persistent setup ----------
            sw_sb = sb.tile([128, 32], F32, tag="sw")
            nc.gpsimd.dma_start(sw_sb[:], sw_d[:])
            sws3 = sb.tile([3, PTS], F16, tag="sws3")
            nc.gpsimd.dma_start(sws3[:], sws3_d[:])
            db3_sb = sb.tile([3, NITER], F32, tag="db3")
            nc.gpsimd.dma_start(db3_sb[:], db3_d[:])

            x_cur = sb.tile([3, PTS], F32, tag="xA")
            nc.gpsimd.dma_start(x_cur[:], x0t_d[:])

            eps_sb = sb.tile([128, 1], F32, tag="epsc")
            nc.vector.memset(eps_sb[:], float(EPS))
            licol = sb.tile([128, NITER], F32, tag="licol")
            nc.vector.memset(licol[:], 0.0)
            a3col = sb.tile([3, NITER], F32, tag="a3col")
            nc.vector.memset(a3col[:], 0.0)

            # zt activation pools (3D tiles [128, NPT, PT])
            zA = [sb.tile([128, NPT, PT], F16, tag=f"zA{co}", name=f"zA{co}")
                  for co in range(8)]
            zB = [sb.tile([128, NPT, PT], F16, tag=f"zB{co}", name=f"zB{co}")
                  for co in range(4)]
            junk16 = sb.tile([128, NPT, PT], F16, tag="junk16")
            scr3 = sb.tile([3, PTS], F16, tag="scr3")
            # aliases: fp16 x for layer0 lives in junk16's first rows (safe:
            # the first junk write of an iter comes after L0's matmuls);
            # tanh preds live in zB[0]'s first rows (zB is free at L7 time).
            xf16 = junk16
            preds16 = zB[0]

            # per-iteration state (rotating buffers across iterations)
            state = {}

            def it_state(it):
                if it in state:
                    return state[it]
                s = {
                    'Ld': sb.tile([11, PTS], F16, tag="Ld", bufs=2,
                                  name=f"Ld_{it}"),
                    'Le': sb.tile([11, PTS], F16, tag="Le", bufs=1,
                                  name=f"Le_{it}"),
                    'r': [sb.tile([13, RP], F16, tag=f"r{b}", bufs=2,
                                  name=f"r_{it}_{b}") for b in range(BL)],
                    'm': sb.tile([128, 32], F32, tag="m_all", bufs=2,
                                 name=f"m_{it}"),
                    'S': sb.tile([128, 32], F32, tag="S", bufs=2,
                                 name=f"S_{it}"),
                }
                state[it] = s
                return s

            def emit_p1(it, t):
                """KNN phase1 tile: d-matmuls + row max for (b, qt)."""
                s = it_state(it)
                b, qt = t // 8, t % 8
                col = t
                qsl = slice(b * N + qt * QT, b * N + (qt + 1) * QT)
                kpd = psk.tile([QT, 2, 512], F32, tag="kpd")
                for rt in range(2):
                    nc.tensor.matmul(kpd[:, rt, :], s['Ld'][0:11, qsl],
                                     s['r'][b][0:11, rt * 512:(rt + 1) * 512],
                                     start=True, stop=True)
                nc.vector.tensor_reduce(s['m'][0:QT, col:col + 1], kpd[:],
                                        AX.XY, OP.max)

            def emit_p4(it, t):
                """KNN phase4 tile: recompute d, compute e, extract."""
                s = it_state(it)
                b, qt = t // 8, t % 8
                col = t
                qsl = slice(b * N + qt * QT, b * N + (qt + 1) * QT)
                kpd = psk.tile([QT, 2, 512], F32, tag="kpd")
                for rt in range(2):
                    nc.tensor.matmul(kpd[:, rt, :], s['Ld'][0:11, qsl],
                                     s['r'][b][0:11, rt * 512:(rt + 1) * 512],
                                     start=True, stop=True)
                kpe = psk.tile([QT, 2, 512], F32, tag="kpd")
                for rt in range(2):
                    nc.tensor.matmul(kpe[:, rt, :], s['Le'][0:11, qsl],
                                     s['r'][b][0:11, rt * 512:(rt + 1) * 512],
                                     start=True, stop=True)
                o16 = sb.tile([QT, 2, 512], F16, tag="o16", bufs=2)
                nc.vector.tensor_scalar(o16[:], kpd[:],
                                        s['m'][0:QT, col:col + 1], None,
                                        OP.is_equal)
                nc.vector.scalar_tensor_tensor(
                    o16[:], kpe[:], 1.0, o16[:], OP.mult, OP.mult,
                    accum_out=s['S'][0:QT, col:col + 1])

            # ---------- main loop ----------
            for it in range(niter):
                s = it_state(it)

                # prefetch: weights, gh, r tiles for this iteration
                wts = []
                for l in range(nlayers):
                    cin, cout = LAYERS[l][:2]
                    wt = sb.tile(
                        [cin, cout] if cin <= 128 else [128, NCI[l] * cout],
                        F16, tag=f"w{l}", name=f"w_{it}_{l}")
                    nc.gpsimd.dma_start(wt[:], w_d[it][l][:])
                    wts.append(wt)
                gh = sb.tile([128, 2, GHTOT], F32, tag="gh", bufs=2)
                nc.gpsimd.dma_start(gh[:], gh_d[it][:])
                if do_knn:
                    for b in range(BL):
                        nc.gpsimd.dma_start(s['r'][b][:], r_d[it][b][:])
                    nc.vector.memset(s['S'][:], 0.0)

                # fp16 copy of x for layer 0 (into junk16 alias rows) --
                # first, so L0's matmuls are not gated on the Ld/Le builds
                nc.vector.tensor_copy(xf16[0:3, :, :], x_cur[:])

                p1c = [0]
                p4c = [0]

                if do_knn:
                    # build Ld(it) = [2x_h, 2x_l, 2x_h, -1, -1]
                    nc.vector.tensor_scalar_mul(s['Ld'][0:3, :], x_cur[:], 2.0)
                    nc.vector.scalar_tensor_tensor(
                        scr3[:], x_cur[:], 2.0, s['Ld'][0:3, :], OP.mult,
                        OP.subtract)
                    nc.gpsimd.dma_start(s['Ld'][3:6, :], scr3[:])
                    nc.gpsimd.dma_start(s['Ld'][6:9, :], s['Ld'][0:3, :])
                    nc.gpsimd.dma_start(s['Ld'][9:11, :], cneg_d[:])

                    if it > 0:
                        # build Le(it-1) from x_cur (post-update coords of
                        # iter it-1), plus term2 for iter it-1
                        sp = it_state(it - 1)
                        nc.vector.tensor_scalar_mul(sp['Le'][0:3, :],
                                                    x_cur[:], -2.0)
                        nc.vector.scalar_tensor_tensor(
                            scr3[:], x_cur[:], -2.0, sp['Le'][0:3, :],
                            OP.mult, OP.subtract)
                        nc.gpsimd.dma_start(sp['Le'][3:6, :], scr3[:])
                        nc.gpsimd.dma_start(sp['Le'][6:9, :], sp['Le'][0:3, :])
                        nc.gpsimd.dma_start(sp['Le'][9:11, :], cpos_d[:])
                        # term2: sum_q sw_q |x|^2 via (x*sqrt(sw))^2
                        nc.vector.tensor_tensor(out=scr3[:], in0=x_cur[:],
                                                in1=sws3[:], op=OP.mult)
                        nc.vector.scalar_tensor_tensor(
                            preds16[0:3, :, :], scr3[:], 1.0, scr3[:],
                            OP.mult, OP.mult,
                            accum_out=a3col[0:3, it - 1:it])

                def gate_col(m_tile, col, gate_ap):
                    # value-preserving write to m[col] that reads gate_ap:
                    # m[col] = 0*gate + m[col].  Pins the KNN batch to the
                    # boundary (scheduler cannot hoist it before the gate).
                    nc.vector.scalar_tensor_tensor(
                        m_tile[0:QT, col:col + 1], gate_ap, 0.0,
                        m_tile[0:QT, col:col + 1], OP.mult, OP.add)

                def knn_batch(bi, gate_ap=None):
                    """Emit the KNN fill slice for boundary bi of iter it."""
                    if not do_knn:
                        return
                    if gate_ap is not None and p1c[0] < 32:
                        gate_col(s['m'], p1c[0], gate_ap)
                    for _ in range(P1_SPLIT[bi]):
                        if p1c[0] < 32:
                            emit_p1(it, p1c[0])
                            p1c[0] += 1
                    if it > 0:
                        if gate_ap is not None and p4c[0] < 32:
                            gate_col(it_state(it - 1)['m'], p4c[0], gate_ap)
                        for _ in range(P4_SPLIT[bi]):
                            if p4c[0] < 32:
                                emit_p4(it - 1, p4c[0])
                                p4c[0] += 1

                # ---------- MLP ----------
                zin = None
                bi = 0
                for l in range(nlayers):
                    cin, cout, has_bn = LAYERS[l]
                    nci, nco = NCI[l], NCO[l]
                    CIP = min(128, cin)
                    zout = zA if ZPOOL[l] == 'A' else zB
                    wt = wts[l]

                    if has_bn:
                        sums = sb.tile([128, nco, NG], F32, tag="sums", bufs=2)
                        ssq = sb.tile([128, nco, NG], F32, tag="ssq", bufs=2)
                        statsr = sb.tile([128, 2, nco], F32, tag="statsr",
                                         bufs=2)

                    for g in range(NG):
                        for co in range(nco):
                            CO = min(128, cout - co * 128)
                            zp = ps.tile([128, 2, 512], F32, tag="zp")
                            for ci in range(nci):
                                if l == 0:
                                    lhs = wt[0:CIP, co * 128:co * 128 + CO]
                                else:
                                    lhs = wt[0:CIP,
                                             ci * cout + co * 128:
                                             ci * cout + co * 128 + CO]
                                for h in range(2):
                                    pt = 2 * g + h
                                    if l == 0:
                                        rhs = xf16[0:3, pt, :]
                                    else:
                                        rhs = zin[ci][0:CIP, pt, :]
                                    nc.tensor.matmul(
                                        zp[0:CO, h, 0:PT], lhs, rhs,
                                        start=(ci == 0), stop=(ci == nci - 1))
                            if has_bn:
                                nc.vector.tensor_scalar(
                                    zout[co][0:CO, 2 * g:2 * g + 2, :],
                                    zp[0:CO, :, 0:PT], 1.0, 0.0, OP.mult,
                                    OP.add,
                                    accum_out=sums[0:CO, co, g:g + 1])
                                # sumsq for this (co, g) slice
                                if co % 2 == 0:
                                    nc.scalar.activation(
                                        junk16[0:CO, 2 * g:2 * g + 2, :],
                                        zout[co][0:CO, 2 * g:2 * g + 2, :],
                                        AF.Square,
                                        accum_out=ssq[0:CO, co, g:g + 1])
                                else:
                                    nc.vector.scalar_tensor_tensor(
                                        junk16[0:CO, 2 * g:2 * g + 2, :],
                                        zout[co][0:CO, 2 * g:2 * g + 2, :],
                                        1.0,
                                        zout[co][0:CO, 2 * g:2 * g + 2, :],
                                        OP.mult, OP.mult,
                                        accum_out=ssq[0:CO, co, g:g + 1])
                            else:
                                nc.scalar.activation(
                                    preds16[0:3, 2 * g:2 * g + 2, :],
                                    zp[0:3, :, 0:PT], AF.Tanh,
                                    bias=db3_sb[:, it:it + 1])

                    if not has_bn:
                        break

                    nc.vector.tensor_reduce(statsr[:, 0, :], sums[:], AX.X,
                                            OP.add)
                    nc.vector.tensor_reduce(statsr[:, 1, :], ssq[:], AX.X,
                                            OP.add)
                    arin = dram.tile([128, 2, nco], F32, tag="arin")
                    arout = dram.tile([128, 2, nco], F32, tag="arout")
                    nc.gpsimd.dma_start(arin[:], statsr[:])
                    if do_ar:
                        nc.gpsimd.collective_compute(
                            "AllReduce", OP.add, replica_groups=RG,
                            ins=[arin.opt()], outs=[arout.opt()])
                    else:
                        nc.gpsimd.dma_start(arout[:], arin[:])

                    # KNN fill while the AllReduce is in flight (gated on
                    # this layer's stats so the scheduler can't hoist it)
                    knn_batch(bi, statsr[0:QT, 0, 0:1])
                    bi += 1

                    statsg = sb.tile([128, 2, nco], F32, tag="statsg", bufs=2)
                    nc.gpsimd.dma_start(statsg[:], arout[:])

                    # affine coefs: a = g/sqrt(var+eps); ct = h - mean*a
                    af = sb.tile([128, 6, nco], F32, tag="af", bufs=2)
                    inv_n = 1.0 / NPTS_GLOBAL
                    nc.vector.tensor_scalar_mul(af[:, 0:2, :], statsg[:],
                                                inv_n)
                    nc.vector.tensor_tensor(out=af[:, 2, :], in0=af[:, 0, :],
                                            in1=af[:, 0, :], op=OP.mult)
                    nc.vector.tensor_tensor(out=af[:, 1, :], in0=af[:, 1, :],
                                            in1=af[:, 2, :], op=OP.subtract)
                    nc.scalar.activation(af[:, 2, :], af[:, 1, :], AF.Sqrt,
                                         bias=eps_sb[:])
                    nc.vector.reciprocal(af[:, 3, :], af[:, 2, :])
                    nc.vector.tensor_tensor(
                        out=af[:, 4, :], in0=gh[:, 0, GHOFF[l]:GHOFF[l] + nco],
                        in1=af[:, 3, :], op=OP.mult)
                    nc.vector.tensor_tensor(out=af[:, 2, :], in0=af[:, 0, :],
                                            in1=af[:, 4, :], op=OP.mult)
                    nc.vector.tensor_tensor(
                        out=af[:, 5, :], in0=gh[:, 1, GHOFF[l]:GHOFF[l] + nco],
                        in1=af[:, 2, :], op=OP.subtract)

                    # affine+relu at (chunk, group) granularity, g-major
                    # so the next layer's first psum chains unblock early;
                    # chunks alternate scalar/vector engines
                    for g in range(NG):
                        for co in range(nco):
                            CO = min(128, cout - co * 128)
                            dst = zout[co][0:CO, 2 * g:2 * g + 2, :]
                            sc = af[0:CO, 4, co:co + 1]
                            bi_ = af[0:CO, 5, co:co + 1]
                            if co % 2 == 0:
                                nc.scalar.activation(dst, dst, AF.Relu,
                                                     bias=bi_, scale=sc)
                            else:
                                nc.vector.tensor_scalar(
                                    dst, dst, sc, bi_, OP.mult, OP.add)
                                nc.vector.tensor_scalar_max(dst, dst, 0.0)
                    zin = zout

                # ---------- x update (in place) ----------
                if nlayers < 8:
                    continue
                nc.vector.tensor_tensor(out=x_cur[:], in0=x_cur[:],
                                        in1=preds16[0:3, :, :], op=OP.add)

                if do_knn and it == niter - 1:
                    # tail: build Le(last)+term2(last), then all of phase4
                    nc.vector.tensor_scalar_mul(s['Le'][0:3, :], x_cur[:],
                                                -2.0)
                    nc.vector.scalar_tensor_tensor(
                        scr3[:], x_cur[:], -2.0, s['Le'][0:3, :], OP.mult,
                        OP.subtract)
                    nc.gpsimd.dma_start(s['Le'][3:6, :], scr3[:])
                    nc.gpsimd.dma_start(s['Le'][6:9, :], s['Le'][0:3, :])
                    nc.gpsimd.dma_start(s['Le'][9:11, :], cpos_d[:])
                    nc.vector.tensor_tensor(out=scr3[:], in0=x_cur[:],
                                            in1=sws3[:], op=OP.mult)
                    nc.vector.scalar_tensor_tensor(
                        preds16[0:3, :, :], scr3[:], 1.0, scr3[:], OP.mult,
                        OP.mult, accum_out=a3col[0:3, it:it + 1])
                    for t in range(32):
                        emit_p4(it, t)

                # loss column for iter it-1 (S complete after its 32 p4 tiles)
                if do_knn and it > 0:
                    sp = it_state(it - 1)
                    jk2 = sb.tile([128, 32], F32, tag="jk2", bufs=2)
                    nc.vector.scalar_tensor_tensor(
                        jk2[:], sp['S'][:], 1.0, sw_sb[:], OP.mult, OP.mult,
                        accum_out=licol[:, it - 1:it])

            if do_knn and niter >= 1 and nlayers >= 8:
                sl = it_state(niter - 1)
                jk2 = sb.tile([128, 32], F32, tag="jk2", bufs=2)
                nc.vector.scalar_tensor_tensor(
                    jk2[:], sl['S'][:], 1.0, sw_sb[:], OP.mult, OP.mult,
                    accum_out=licol[:, niter - 1:niter])

            lacc = sb.tile([128, 1], F32, tag="laccA")
            nc.vector.tensor_reduce(lacc[:], licol[:], AX.X, OP.add)
            acc3f = sb.tile([3, 1], F32, tag="acc3A")
            nc.vector.tensor_reduce(acc3f[:], a3col[0:3, :], AX.X, OP.add)
            nc.gpsimd.dma_start(loss_d[:], lacc[:])
            nc.gpsimd.dma_start(loss3_d[:], acc3f[:])
    nc.compile()
    return nc


def _host_prep(inputs):
    """Build per-core input maps."""
    f32 = np.float32
    noisy = np.asarray(inputs["pcl_noisy"], f32)
    clean = np.asarray(inputs["pcl_clean"], f32)
    seeds = np.asarray(inputs["pcl_seeds"], f32)
    std = np.asarray(inputs["pcl_std"], f32)
    noise = np.asarray(inputs["noise"], f32)

    pn = noisy - seeds
    pc = clean - seeds
    sdist = np.sum(pn.astype(np.float64) ** 2, -1, keepdims=True)
    max_sq = sdist[:, -1:, :]
    sw = np.exp(-sdist * 9.0 / max_sq)[..., 0]
    sw = (sw / sw.sum(1, keepdims=True))  # [B, N] float64

    tgts = []
    cur = std.copy()
    for i in range(NITER):
        if i < NITER - 1:
            cur = cur / NOISE_DECAY
            tgts.append(pc + noise[i] * cur[:, None, None])
        else:
            tgts.append(pc.copy())

    sent = np.full((RP - N, 3), 100.0, np.float64)

    shared = {}
    for i in range(NITER):
        for l in range(8):
            key = f'ew{l+1}' if l < 5 else f'dw{l-4}'
            w = np.asarray(inputs[key], f32)[i].astype(np.float16)
            cin, cout = LAYERS[l][:2]
            if cin <= 128:
                shared[f"w_{i}_{l}"] = w
            else:
                nci = NCI[l]
                shared[f"w_{i}_{l}"] = np.ascontiguousarray(
                    w.reshape(nci, 128, cout).transpose(1, 0, 2).reshape(
                        128, nci * cout))
        gharr = np.zeros((128, 2, GHTOT), f32)
        for l in range(7):
            nco = NCO[l]
            cout = LAYERS[l][1]
            gk = f'eg{l+1}' if l < 5 else f'dg{l-4}'
            hk = f'eh{l+1}' if l < 5 else f'dh{l-4}'
            g = np.asarray(inputs[gk], f32)[i]
            h = np.asarray(inputs[hk], f32)[i]
            gp = np.zeros(nco * 128, f32); gp[:cout] = g
            hp = np.zeros(nco * 128, f32); hp[:cout] = h
            gharr[:, 0, GHOFF[l]:GHOFF[l] + nco] = gp.reshape(nco, 128).T
            gharr[:, 1, GHOFF[l]:GHOFF[l] + nco] = hp.reshape(nco, 128).T
        shared[f"gh_{i}"] = gharr
    shared["db3t"] = np.ascontiguousarray(np.asarray(inputs["db3"], f32).T)
    shared["cneg"] = np.full((2, PTS), -1.0, np.float16)
    shared["cpos"] = np.full((2, PTS), 1.0, np.float16)

    in_maps = []
    for c in range(NCORES):
        bs = slice(c * BL, (c + 1) * BL)
        m = dict(shared)
        m["x0t"] = np.ascontiguousarray(
            pn[bs].transpose(2, 0, 1).reshape(3, PTS))
        swc = np.zeros((128, 32), f32)
        for b in range(BL):
            for qt in range(8):
                swc[0:QT, b * 8 + qt] = sw[c * BL + b,
                                           qt * QT:(qt + 1) * QT].astype(f32)
        m["sw"] = swc
        m["sws3"] = np.broadcast_to(
            np.sqrt(sw[bs]).reshape(1, PTS), (3, PTS)).astype(np.float16)
        for i in range(NITER):
            for b in range(BL):
                coords = np.concatenate(
                    [tgts[i][c * BL + b].astype(np.float64), sent], 0)
                rh = coords.astype(np.float16)
                rl = (coords - rh.astype(np.float64)).astype(np.float16)
                rsq = (coords ** 2).sum(1)
                rsqh = rsq.astype(np.float16)
                rsql = (rsq - rsqh.astype(np.float64)).astype(np.float16)
                R = np.empty((13, RP), np.float16)
                R[0:3] = rh.T
                R[3:6] = rh.T
                R[6:9] = rl.T
                R[9] = rsqh
                R[10] = rsql
                R[11] = 1.0
                R[12] = 1.0
                m[f"rknn_{i}_{b}"] = R
        in_maps.append(m)
    return in_maps


def kernel(**inputs):
    if "nc" not in _NC_CACHE:
        _NC_CACHE["nc"] = _build()
    nc = _NC_CACHE["nc"]
    in_maps = _host_prep(inputs)
    res = run_bass_kernel_spmd(nc, in_maps, list(range(NCORES))).results
    total = 0.0
    for c in range(NCORES):
        total += float(res[c]["loss_part"].sum())
        total += float(res[c]["loss_part3"].sum())
    return np.asarray(total / B, dtype=np.float32)


# revision 20
# speedup vs baseline: 1.0409x; 1.0012x over previous
"""DenoiseNet loss kernel for 8 Trainium2 NeuronCores.

Strategy: pure data parallel over the batch (4 batches/core). PointNet MLP in
fp16 (fp32 PSUM accumulate), exact global BatchNorm via per-layer AllReduce of
(sum, sumsq). KNN argmin via hi/lo-fp16-split matmul of 2q.r-|r|^2, extraction
of the matched |v-r*|^2 by is_equal one-hot + fused multiply-reduce.

v2 restructure for PE warmth / engine balance:
 - weights host-packed per layer, one DMA per (iter, layer), prefetched at
   iteration start.
 - PSUM evictions on the vector engine with fused sum-accumulate; sumsq
   pass split between scalar/vector; BN affine+ReLU fused into a single
   scalar-engine activation per 128-channel chunk (true per-channel scale,
   no weight folding).
 - KNN matmuls of phase1(iter i) and phase4(iter i-1) are interleaved into
   the 7 BN-AllReduce stall windows of iter i, keeping the PE busy (HAM
   stays at full clock) while the collective is in flight.
"""
import numpy as np

import concourse.bass as bass
import concourse.mybir as mybir
import concourse.tile as tile
from concourse import bacc
from concourse.bass_utils import run_bass_kernel_spmd

dt = mybir.dt
F32 = dt.float32
F16 = dt.float16
AF = mybir.ActivationFunctionType
OP = mybir.AluOpType
AX = mybir.AxisListType

B, N, NCORES = 32, 1000, 8
BL = B // NCORES            # 4 batches per core
PTS = BL * N                # 4000 points per core
NITER = 4
NPTS_GLOBAL = B * N         # 32000 (BN population)
EPS = 1e-5
NOISE_DECAY = 4.0
QT = 125                    # q tile (8 per batch)
RP = 1024                   # padded ref points (24 sentinels)
PT = 500                    # pts tile (8 tiles of 500)
NPT = PTS // PT             # 8
NG = NPT // 2               # 4 psum groups of 2 tiles

# (C_in, C_out, has_bn) per layer
LAYERS = [(3, 64, 1), (64, 128, 1), (128, 256, 1), (256, 512, 1),
          (512, 1024, 1), (1024, 512, 1), (512, 256, 1), (256, 3, 0)]
NCI = [max(1, ci // 128) for ci, co, _ in LAYERS]
NCO = [max(1, (co + 127) // 128) for ci, co, _ in LAYERS]
# gh packing offsets for the 7 BN layers
GHOFF = [0]
for _l in range(7):
    GHOFF.append(GHOFF[-1] + NCO[_l])
GHTOT = GHOFF[-1]           # 22

# zt buffer assignment: even layers write pool A (8 bufs), odd write pool B (4)
ZPOOL = ['A', 'B', 'A', 'B', 'A', 'B', 'A', None]

# KNN fill distribution: tiles emitted at each of the 7 BN boundaries
P1_SPLIT = [5, 5, 5, 5, 4, 4, 4]
P4_SPLIT = [5, 5, 5, 5, 4, 4, 4]

RG = [list(range(NCORES))]

_NC_CACHE = {}


def _build(niter=NITER, nlayers=8, do_knn=True, do_ar=True):
    nc = bacc.Bacc(None, target_bir_lowering=False, debug=False)

    x0t_d = nc.dram_tensor("x0t", [3, PTS], F32, kind="ExternalInput")
    cneg_d = nc.dram_tensor("cneg", [2, PTS], F16, kind="ExternalInput")
    cpos_d = nc.dram_tensor("cpos", [2, PTS], F16, kind="ExternalInput")
    sw_d = nc.dram_tensor("sw", [128, 32], F32, kind="ExternalInput")
    sws3_d = nc.dram_tensor("sws3", [3, PTS], F16, kind="ExternalInput")
    db3_d = nc.dram_tensor("db3t", [3, NITER], F32, kind="ExternalInput")
    r_d = [[nc.dram_tensor(f"rknn_{i}_{b}", [13, RP], F16, kind="ExternalInput")
            for b in range(BL)] for i in range(NITER)]
    w_d = [[nc.dram_tensor(
        f"w_{i}_{l}",
        [LAYERS[l][0], LAYERS[l][1]] if LAYERS[l][0] <= 128
        else [128, NCI[l] * LAYERS[l][1]], F16, kind="ExternalInput")
        for l in range(8)] for i in range(NITER)]
    gh_d = [nc.dram_tensor(f"gh_{i}", [128, 2, GHTOT], F32,
                           kind="ExternalInput") for i in range(NITER)]
    loss_d = nc.dram_tensor("loss_part", [128, 1], F32, kind="ExternalOutput")
    loss3_d = nc.dram_tensor("loss_part3", [3, 1], F32, kind="ExternalOutput")

    with tile.TileContext(nc) as tc:
        with (
            tc.tile_pool(name="sb", bufs=1) as sb,
            tc.tile_pool(name="ps", bufs=2, space="PSUM") as ps,
            tc.tile_pool(name="psk", bufs=2, space="PSUM") as psk,
            tc.tile_pool(name="dram", bufs=2, space="DRAM") as dram,
        ):
            # ---------- persistent setup ----------
            sw_sb = sb.tile([128, 32], F32, tag="sw")
            nc.gpsimd.dma_start(sw_sb[:], sw_d[:])
            sws3 = sb.tile([3, PTS], F16, tag="sws3")
            nc.gpsimd.dma_start(sws3[:], sws3_d[:])
            db3_sb = sb.tile([3, NITER], F32, tag="db3")
            nc.gpsimd.dma_start(db3_sb[:], db3_d[:])

            x_cur = sb.tile([3, PTS], F32, tag="xA")
            nc.gpsimd.dma_start(x_cur[:], x0t_d[:])

            eps_sb = sb.tile([128, 1], F32, tag="epsc")
            nc.vector.memset(eps_sb[:], float(EPS))
            licol = sb.tile([128, NITER], F32, tag="licol")
            nc.vector.memset(licol[:], 0.0)
            a3col = sb.tile([3, NITER], F32, tag="a3col")
            nc.vector.memset(a3col[:], 0.0)

            # zt activation pools (3D tiles [128, NPT, PT])
            zA = [sb.tile([128, NPT, PT], F16, tag=f"zA{co}", name=f"zA{co}")
                  for co in range(8)]
            zB = [sb.tile([128, NPT, PT], F16, tag=f"zB{co}", name=f"zB{co}")
                  for co in range(4)]
            junk16 = sb.tile([128, NPT, PT], F16, tag="junk16")
            scr3 = sb.tile([3, PTS], F16, tag="scr3")
            # aliases: fp16 x for layer0 lives in junk16's first rows (safe:
            # the first junk write of an iter comes after L0's matmuls);
            # tanh preds live in zB[0]'s first rows (zB is free at L7 time).
            xf16 = junk16
            preds16 = zB[0]

            # per-iteration state (rotating buffers across iterations)
            state = {}

            def it_state(it):
                if it in state:
                    return state[it]
                s = {
                    'Ld': sb.tile([11, PTS], F16, tag="Ld", bufs=2,
                                  name=f"Ld_{it}"),
                    'Le': sb.tile([11, PTS], F16, tag="Le", bufs=1,
                                  name=f"Le_{it}"),
                    'r': [sb.tile([13, RP], F16, tag=f"r{b}", bufs=2,
                                  name=f"r_{it}_{b}") for b in range(BL)],
                    'm': sb.tile([128, 32], F32, tag="m_all", bufs=2,
                                 name=f"m_{it}"),
                    'S': sb.tile([128, 32], F32, tag="S", bufs=2,
                                 name=f"S_{it}"),
                }
                state[it] = s
                return s

            def emit_p1(it, t):
                """KNN phase1 tile: d-matmuls + row max for (b, qt)."""
                s = it_state(it)
                b, qt = t // 8, t % 8
                col = t
                qsl = slice(b * N + qt * QT, b * N + (qt + 1) * QT)
                kpd = psk.tile([QT, 2, 512], F32, tag="kpd")
                for rt in range(2):
                    nc.tensor.matmul(kpd[:, rt, :], s['Ld'][0:11, qsl],
                                     s['r'][b][0:11, rt * 512:(rt + 1) * 512],
                                     start=True, stop=True)
                nc.vector.tensor_reduce(s['m'][0:QT, col:col + 1], kpd[:],
                                        AX.XY, OP.max)

            def emit_p4(it, t):
                """KNN phase4 tile: recompute d, compute e, extract."""
                s = it_state(it)
                b, qt = t // 8, t % 8
                col = t
                qsl = slice(b * N + qt * QT, b * N + (qt + 1) * QT)
                kpd = psk.tile([QT, 2, 512], F32, tag="kpd")
                for rt in range(2):
                    nc.tensor.matmul(kpd[:, rt, :], s['Ld'][0:11, qsl],
                                     s['r'][b][0:11, rt * 512:(rt + 1) * 512],
                                     start=True, stop=True)
                kpe = psk.tile([QT, 2, 512], F32, tag="kpd")
                for rt in range(2):
                    nc.tensor.matmul(kpe[:, rt, :], s['Le'][0:11, qsl],
                                     s['r'][b][0:11, rt * 512:(rt + 1) * 512],
                                     start=True, stop=True)
                o16 = sb.tile([QT, 2, 512], F16, tag="o16", bufs=2)
                nc.vector.tensor_scalar(o16[:], kpd[:],
                                        s['m'][0:QT, col:col + 1], None,
                                        OP.is_equal)
                nc.vector.scalar_tensor_tensor(
                    o16[:], kpe[:], 1.0, o16[:], OP.mult, OP.mult,
                    accum_out=s['S'][0:QT, col:col + 1])

            # ---------- main loop ----------
            for it in range(niter):
                s = it_state(it)

                # prefetch: weights, gh, r tiles for this iteration
                wts = []
                for l in range(nlayers):
                    cin, cout = LAYERS[l][:2]
                    wt = sb.tile(
                        [cin, cout] if cin <= 128 else [128, NCI[l] * cout],
                        F16, tag=f"w{l}", name=f"w_{it}_{l}")
                    nc.gpsimd.dma_start(wt[:], w_d[it][l][:])
                    wts.append(wt)
                gh = sb.tile([128, 2, GHTOT], F32, tag="gh", bufs=2)
                nc.gpsimd.dma_start(gh[:], gh_d[it][:])
                if do_knn:
                    for b in range(BL):
                        nc.gpsimd.dma_start(s['r'][b][:], r_d[it][b][:])
                    nc.vector.memset(s['S'][:], 0.0)

                # fp16 copy of x for layer 0 (into junk16 alias rows) --
                # first, so L0's matmuls are not gated on the Ld/Le builds
                nc.vector.tensor_copy(xf16[0:3, :, :], x_cur[:])

                p1c = [0]
                p4c = [0]

                if do_knn:
                    # build Ld(it) = [2x_h, 2x_l, 2x_h, -1, -1]
                    nc.vector.tensor_scalar_mul(s['Ld'][0:3, :], x_cur[:], 2.0)
                    nc.vector.scalar_tensor_tensor(
                        scr3[:], x_cur[:], 2.0, s['Ld'][0:3, :], OP.mult,
                        OP.subtract)
                    nc.gpsimd.dma_start(s['Ld'][3:6, :], scr3[:])
                    nc.gpsimd.dma_start(s['Ld'][6:9, :], s['Ld'][0:3, :])
                    nc.gpsimd.dma_start(s['Ld'][9:11, :], cneg_d[:])

                    if it > 0:
                        # build Le(it-1) from x_cur (post-update coords of
                        # iter it-1), plus term2 for iter it-1
                        sp = it_state(it - 1)
                        nc.vector.tensor_scalar_mul(sp['Le'][0:3, :],
                                                    x_cur[:], -2.0)
                        nc.vector.scalar_tensor_tensor(
                            scr3[:], x_cur[:], -2.0, sp['Le'][0:3, :],
                            OP.mult, OP.subtract)
                        nc.gpsimd.dma_start(sp['Le'][3:6, :], scr3[:])
                        nc.gpsimd.dma_start(sp['Le'][6:9, :], sp['Le'][0:3, :])
                        nc.gpsimd.dma_start(sp['Le'][9:11, :], cpos_d[:])
                        # term2: sum_q sw_q |x|^2 via (x*sqrt(sw))^2
                        nc.vector.tensor_tensor(out=scr3[:], in0=x_cur[:],
                                                in1=sws3[:], op=OP.mult)
                        nc.vector.scalar_tensor_tensor(
                            preds16[0:3, :, :], scr3[:], 1.0, scr3[:],
                            OP.mult, OP.mult,
                            accum_out=a3col[0:3, it - 1:it])

                def gate_col(m_tile, col, gate_ap):
                    # value-preserving write to m[col] that reads gate_ap:
                    # m[col] = 0*gate + m[col].  Pins the KNN batch to the
                    # boundary (scheduler cannot hoist it before the gate).
                    nc.vector.scalar_tensor_tensor(
                        m_tile[0:QT, col:col + 1], gate_ap, 0.0,
                        m_tile[0:QT, col:col + 1], OP.mult, OP.add)

                def knn_batch(bi, gate_ap=None):
                    """Emit the KNN fill slice for boundary bi of iter it."""
                    if not do_knn:
                        return
                    if gate_ap is not None and p1c[0] < 32:
                        gate_col(s['m'], p1c[0], gate_ap)
                    for _ in range(P1_SPLIT[bi]):
                        if p1c[0] < 32:
                            emit_p1(it, p1c[0])
                            p1c[0] += 1
                    if it > 0:
                        if gate_ap is not None and p4c[0] < 32:
                            gate_col(it_state(it - 1)['m'], p4c[0], gate_ap)
                        for _ in range(P4_SPLIT[bi]):
                            if p4c[0] < 32:
                                emit_p4(it - 1, p4c[0])
                                p4c[0] += 1

                # ---------- MLP ----------
                zin = None
                bi = 0
                for l in range(nlayers):
                    cin, cout, has_bn = LAYERS[l]
                    nci, nco = NCI[l], NCO[l]
                    CIP = min(128, cin)
                    zout = zA if ZPOOL[l] == 'A' else zB
                    wt = wts[l]

                    if has_bn:
                        sums = sb.tile([128, nco, NG], F32, tag="sums", bufs=2)
                        ssq = sb.tile([128, nco, NG], F32, tag="ssq", bufs=2)
                        statsr = sb.tile([128, 2, nco], F32, tag="statsr",
                                         bufs=2)

                    for g in range(NG):
                        for co in range(nco):
                            CO = min(128, cout - co * 128)
                            zp = ps.tile([128, 2, 512], F32, tag="zp")
                            for ci in range(nci):
                                if l == 0:
                                    lhs = wt[0:CIP, co * 128:co * 128 + CO]
                                else:
                                    lhs = wt[0:CIP,
                                             ci * cout + co * 128:
                                             ci * cout + co * 128 + CO]
                                for h in range(2):
                                    pt = 2 * g + h
                                    if l == 0:
                                        rhs = xf16[0:3, pt, :]
                                    else:
                                        rhs = zin[ci][0:CIP, pt, :]
                                    nc.tensor.matmul(
                                        zp[0:CO, h, 0:PT], lhs, rhs,
                                        start=(ci == 0), stop=(ci == nci - 1))
                            if has_bn:
                                nc.vector.tensor_scalar(
                                    zout[co][0:CO, 2 * g:2 * g + 2, :],
                                    zp[0:CO, :, 0:PT], 1.0, 0.0, OP.mult,
                                    OP.add,
                                    accum_out=sums[0:CO, co, g:g + 1])
                                # sumsq for this (co, g) slice
                                if co % 2 == 0:
                                    nc.scalar.activation(
                                        junk16[0:CO, 2 * g:2 * g + 2, :],
                                        zout[co][0:CO, 2 * g:2 * g + 2, :],
                                        AF.Square,
                                        accum_out=ssq[0:CO, co, g:g + 1])
                                else:
                                    nc.vector.scalar_tensor_tensor(
                                        junk16[0:CO, 2 * g:2 * g + 2, :],
                                        zout[co][0:CO, 2 * g:2 * g + 2, :],
                                        1.0,
                                        zout[co][0:CO, 2 * g:2 * g + 2, :],
                                        OP.mult, OP.mult,
                                        accum_out=ssq[0:CO, co, g:g + 1])
                            else:
                                nc.scalar.activation(
                                    preds16[0:3, 2 * g:2 * g + 2, :],
                                    zp[0:3, :, 0:PT], AF.Tanh,
                                    bias=db3_sb[:, it:it + 1])

                    if not has_bn:
                        break

                    nc.vector.tensor_reduce(statsr[:, 0, :], sums[:], AX.X,
                                            OP.add)
                    nc.vector.tensor_reduce(statsr[:, 1, :], ssq[:], AX.X,
                                            OP.add)
                    arin = dram.tile([128, 2, nco], F32, tag="arin")
                    arout = dram.tile([128, 2, nco], F32, tag="arout")
                    nc.gpsimd.dma_start(arin[:], statsr[:])
                    if do_ar:
                        nc.gpsimd.collective_compute(
                            "AllReduce", OP.add, replica_groups=RG,
                            ins=[arin.opt()], outs=[arout.opt()])
                    else:
                        nc.gpsimd.dma_start(arout[:], arin[:])

                    statsg = sb.tile([128, 2, nco], F32, tag="statsg", bufs=2)
                    nc.gpsimd.dma_start(statsg[:], arout[:])

                    # affine coefs: a = g/sqrt(var+eps); ct = h - mean*a
                    af = sb.tile([128, 6, nco], F32, tag="af", bufs=2)
                    inv_n = 1.0 / NPTS_GLOBAL
                    nc.vector.tensor_scalar_mul(af[:, 0:2, :], statsg[:],
                                                inv_n)
                    nc.vector.tensor_tensor(out=af[:, 2, :], in0=af[:, 0, :],
                                            in1=af[:, 0, :], op=OP.mult)
                    nc.vector.tensor_tensor(out=af[:, 1, :], in0=af[:, 1, :],
                                            in1=af[:, 2, :], op=OP.subtract)
                    nc.scalar.activation(af[:, 2, :], af[:, 1, :], AF.Sqrt,
                                         bias=eps_sb[:])
                    nc.vector.reciprocal(af[:, 3, :], af[:, 2, :])
                    nc.vector.tensor_tensor(
                        out=af[:, 4, :], in0=gh[:, 0, GHOFF[l]:GHOFF[l] + nco],
                        in1=af[:, 3, :], op=OP.mult)
                    nc.vector.tensor_tensor(out=af[:, 2, :], in0=af[:, 0, :],
                                            in1=af[:, 4, :], op=OP.mult)
                    nc.vector.tensor_tensor(
                        out=af[:, 5, :], in0=gh[:, 1, GHOFF[l]:GHOFF[l] + nco],
                        in1=af[:, 2, :], op=OP.subtract)

                    # affine+relu at (chunk, group) granularity, g-major
                    # so the next layer's first psum chains unblock early;
                    # chunks alternate scalar/vector engines
                    for g in range(NG):
                        for co in range(nco):
                            CO = min(128, cout - co * 128)
                            dst = zout[co][0:CO, 2 * g:2 * g + 2, :]
                            sc = af[0:CO, 4, co:co + 1]
                            bi_ = af[0:CO, 5, co:co + 1]
                            if co % 2 == 0:
                                nc.scalar.activation(dst, dst, AF.Relu,
                                                     bias=bi_, scale=sc)
                            else:
                                nc.vector.tensor_scalar(
                                    dst, dst, sc, bi_, OP.mult, OP.add)
                                nc.vector.tensor_scalar_max(dst, dst, 0.0)
                    # KNN fill for the AllReduce window, emitted after the
                    # coef/affine chain so those win vector-engine priority;
                    # the stats gate still pins the fill to this boundary.
                    knn_batch(bi, statsr[0:QT, 0, 0:1])
                    bi += 1
                    zin = zout

                # ---------- x update (in place) ----------
                if nlayers < 8:
                    continue
                nc.vector.tensor_tensor(out=x_cur[:], in0=x_cur[:],
                                        in1=preds16[0:3, :, :], op=OP.add)

                if do_knn and it == niter - 1:
                    # tail: build Le(last)+term2(last), then all of phase4
                    nc.vector.tensor_scalar_mul(s['Le'][0:3, :], x_cur[:],
                                                -2.0)
                    nc.vector.scalar_tensor_tensor(
                        scr3[:], x_cur[:], -2.0, s['Le'][0:3, :], OP.mult,
                        OP.subtract)
                    nc.gpsimd.dma_start(s['Le'][3:6, :], scr3[:])
                    nc.gpsimd.dma_start(s['Le'][6:9, :], s['Le'][0:3, :])
                    nc.gpsimd.dma_start(s['Le'][9:11, :], cpos_d[:])
                    nc.vector.tensor_tensor(out=scr3[:], in0=x_cur[:],
                                            in1=sws3[:], op=OP.mult)
                    nc.vector.scalar_tensor_tensor(
                        preds16[0:3, :, :], scr3[:], 1.0, scr3[:], OP.mult,
                        OP.mult, accum_out=a3col[0:3, it:it + 1])
                    for t in range(32):
                        emit_p4(it, t)

                # loss column for iter it-1 (S complete after its 32 p4 tiles)
                if do_knn and it > 0:
                    sp = it_state(it - 1)
                    jk2 = sb.tile([128, 32], F32, tag="jk2", bufs=2)
                    nc.vector.scalar_tensor_tensor(
                        jk2[:], sp['S'][:], 1.0, sw_sb[:], OP.mult, OP.mult,
                        accum_out=licol[:, it - 1:it])

            if do_knn and niter >= 1 and nlayers >= 8:
                sl = it_state(niter - 1)
                jk2 = sb.tile([128, 32], F32, tag="jk2", bufs=2)
                nc.vector.scalar_tensor_tensor(
                    jk2[:], sl['S'][:], 1.0, sw_sb[:], OP.mult, OP.mult,
                    accum_out=licol[:, niter - 1:niter])

            lacc = sb.tile([128, 1], F32, tag="laccA")
            nc.vector.tensor_reduce(lacc[:], licol[:], AX.X, OP.add)
            acc3f = sb.tile([3, 1], F32, tag="acc3A")
            nc.vector.tensor_reduce(acc3f[:], a3col[0:3, :], AX.X, OP.add)
            nc.gpsimd.dma_start(loss_d[:], lacc[:])
            nc.gpsimd.dma_start(loss3_d[:], acc3f[:])
    nc.compile()
    return nc


def _host_prep(inputs):
    """Build per-core input maps."""
    f32 = np.float32
    noisy = np.asarray(inputs["pcl_noisy"], f32)
    clean = np.asarray(inputs["pcl_clean"], f32)
    seeds = np.asarray(inputs["pcl_seeds"], f32)
    std = np.asarray(inputs["pcl_std"], f32)
    noise = np.asarray(inputs["noise"], f32)

    pn = noisy - seeds
    pc = clean - seeds
    sdist = np.sum(pn.astype(np.float64) ** 2, -1, keepdims=True)
    max_sq = sdist[:, -1:, :]
    sw = np.exp(-sdist * 9.0 / max_sq)[..., 0]
    sw = (sw / sw.sum(1, keepdims=True))  # [B, N] float64

    tgts = []
    cur = std.copy()
    for i in range(NITER):
        if i < NITER - 1:
            cur = cur / NOISE_DECAY
            tgts.append(pc + noise[i] * cur[:, None, None])
        else:
            tgts.append(pc.copy())

    sent = np.full((RP - N, 3), 100.0, np.float64)

    shared = {}
    for i in range(NITER):
        for l in range(8):
            key = f'ew{l+1}' if l < 5 else f'dw{l-4}'
            w = np.asarray(inputs[key], f32)[i].astype(np.float16)
            cin, cout = LAYERS[l][:2]
            if cin <= 128:
                shared[f"w_{i}_{l}"] = w
            else:
                nci = NCI[l]
                shared[f"w_{i}_{l}"] = np.ascontiguousarray(
                    w.reshape(nci, 128, cout).transpose(1, 0, 2).reshape(
                        128, nci * cout))
        gharr = np.zeros((128, 2, GHTOT), f32)
        for l in range(7):
            nco = NCO[l]
            cout = LAYERS[l][1]
            gk = f'eg{l+1}' if l < 5 else f'dg{l-4}'
            hk = f'eh{l+1}' if l < 5 else f'dh{l-4}'
            g = np.asarray(inputs[gk], f32)[i]
            h = np.asarray(inputs[hk], f32)[i]
            gp = np.zeros(nco * 128, f32); gp[:cout] = g
            hp = np.zeros(nco * 128, f32); hp[:cout] = h
            gharr[:, 0, GHOFF[l]:GHOFF[l] + nco] = gp.reshape(nco, 128).T
            gharr[:, 1, GHOFF[l]:GHOFF[l] + nco] = hp.reshape(nco, 128).T
        shared[f"gh_{i}"] = gharr
    shared["db3t"] = np.ascontiguousarray(np.asarray(inputs["db3"], f32).T)
    shared["cneg"] = np.full((2, PTS), -1.0, np.float16)
    shared["cpos"] = np.full((2, PTS), 1.0, np.float16)

    in_maps = []
    for c in range(NCORES):
        bs = slice(c * BL, (c + 1) * BL)
        m = dict(shared)
        m["x0t"] = np.ascontiguousarray(
            pn[bs].transpose(2, 0, 1).reshape(3, PTS))
        swc = np.zeros((128, 32), f32)
        for b in range(BL):
            for qt in range(8):
                swc[0:QT, b * 8 + qt] = sw[c * BL + b,
                                           qt * QT:(qt + 1) * QT].astype(f32)
        m["sw"] = swc
        m["sws3"] = np.broadcast_to(
            np.sqrt(sw[bs]).reshape(1, PTS), (3, PTS)).astype(np.float16)
        for i in range(NITER):
            for b in range(BL):
                coords = np.concatenate(
                    [tgts[i][c * BL + b].astype(np.float64), sent], 0)
                rh = coords.astype(np.float16)
                rl = (coords - rh.astype(np.float64)).astype(np.float16)
                rsq = (coords ** 2).sum(1)
                rsqh = rsq.astype(np.float16)
                rsql = (rsq - rsqh.astype(np.float64)).astype(np.float16)
                R = np.empty((13, RP), np.float16)
                R[0:3] = rh.T
                R[3:6] = rh.T
                R[6:9] = rl.T
                R[9] = rsqh
                R[10] = rsql
                R[11] = 1.0
                R[12] = 1.0
                m[f"rknn_{i}_{b}"] = R
        in_maps.append(m)
    return in_maps


def kernel(**inputs):
    if "nc" not in _NC_CACHE:
        _NC_CACHE["nc"] = _build()
    nc = _NC_CACHE["nc"]
    in_maps = _host_prep(inputs)
    res = run_bass_kernel_spmd(nc, in_maps, list(range(NCORES))).results
    total = 0.0
    for c in range(NCORES):
        total += float(res[c]["loss_part"].sum())
        total += float(res[c]["loss_part3"].sum())
    return np.asarray(total / B, dtype=np.float32)


# revision 21
# speedup vs baseline: 1.0428x; 1.0017x over previous
"""DenoiseNet loss kernel for 8 Trainium2 NeuronCores.

Strategy: pure data parallel over the batch (4 batches/core). PointNet MLP in
fp16 (fp32 PSUM accumulate), exact global BatchNorm via per-layer AllReduce of
(sum, sumsq). KNN argmin via hi/lo-fp16-split matmul of 2q.r-|r|^2, extraction
of the matched |v-r*|^2 by is_equal one-hot + fused multiply-reduce.

v2 restructure for PE warmth / engine balance:
 - weights host-packed per layer, one DMA per (iter, layer), prefetched at
   iteration start.
 - PSUM evictions on the vector engine with fused sum-accumulate; sumsq
   pass split between scalar/vector; BN affine+ReLU fused into a single
   scalar-engine activation per 128-channel chunk (true per-channel scale,
   no weight folding).
 - KNN matmuls of phase1(iter i) and phase4(iter i-1) are interleaved into
   the 7 BN-AllReduce stall windows of iter i, keeping the PE busy (HAM
   stays at full clock) while the collective is in flight.
"""
import numpy as np

import concourse.bass as bass
import concourse.mybir as mybir
import concourse.tile as tile
from concourse import bacc
from concourse.bass_utils import run_bass_kernel_spmd

dt = mybir.dt
F32 = dt.float32
F16 = dt.float16
AF = mybir.ActivationFunctionType
OP = mybir.AluOpType
AX = mybir.AxisListType

B, N, NCORES = 32, 1000, 8
BL = B // NCORES            # 4 batches per core
PTS = BL * N                # 4000 points per core
NITER = 4
NPTS_GLOBAL = B * N         # 32000 (BN population)
EPS = 1e-5
NOISE_DECAY = 4.0
QT = 125                    # q tile (8 per batch)
RP = 1024                   # padded ref points (24 sentinels)
PT = 500                    # pts tile (8 tiles of 500)
NPT = PTS // PT             # 8
NG = NPT // 2               # 4 psum groups of 2 tiles

# (C_in, C_out, has_bn) per layer
LAYERS = [(3, 64, 1), (64, 128, 1), (128, 256, 1), (256, 512, 1),
          (512, 1024, 1), (1024, 512, 1), (512, 256, 1), (256, 3, 0)]
NCI = [max(1, ci // 128) for ci, co, _ in LAYERS]
NCO = [max(1, (co + 127) // 128) for ci, co, _ in LAYERS]
# gh packing offsets for the 7 BN layers
GHOFF = [0]
for _l in range(7):
    GHOFF.append(GHOFF[-1] + NCO[_l])
GHTOT = GHOFF[-1]           # 22

# zt buffer assignment: even layers write pool A (8 bufs), odd write pool B (4)
ZPOOL = ['A', 'B', 'A', 'B', 'A', 'B', 'A', None]

# KNN fill distribution: tiles emitted at each of the 7 BN boundaries
P1_SPLIT = [5, 5, 5, 5, 4, 4, 4]
P4_SPLIT = [5, 5, 5, 5, 4, 4, 4]

RG = [list(range(NCORES))]

_NC_CACHE = {}


def _build(niter=NITER, nlayers=8, do_knn=True, do_ar=True):
    nc = bacc.Bacc(None, target_bir_lowering=False, debug=False)

    x0t_d = nc.dram_tensor("x0t", [3, PTS], F32, kind="ExternalInput")
    cneg_d = nc.dram_tensor("cneg", [2, PTS], F16, kind="ExternalInput")
    cpos_d = nc.dram_tensor("cpos", [2, PTS], F16, kind="ExternalInput")
    sw_d = nc.dram_tensor("sw", [128, 32], F32, kind="ExternalInput")
    sws3_d = nc.dram_tensor("sws3", [3, PTS], F16, kind="ExternalInput")
    db3_d = nc.dram_tensor("db3t", [3, NITER], F32, kind="ExternalInput")
    r_d = [[nc.dram_tensor(f"rknn_{i}_{b}", [13, RP], F16, kind="ExternalInput")
            for b in range(BL)] for i in range(NITER)]
    w_d = [[nc.dram_tensor(
        f"w_{i}_{l}",
        [LAYERS[l][0], LAYERS[l][1]] if LAYERS[l][0] <= 128
        else [128, NCI[l] * LAYERS[l][1]], F16, kind="ExternalInput")
        for l in range(8)] for i in range(NITER)]
    gh_d = [nc.dram_tensor(f"gh_{i}", [128, 2, GHTOT], F32,
                           kind="ExternalInput") for i in range(NITER)]
    loss_d = nc.dram_tensor("loss_part", [128, 1], F32, kind="ExternalOutput")
    loss3_d = nc.dram_tensor("loss_part3", [3, 1], F32, kind="ExternalOutput")

    with tile.TileContext(nc) as tc:
        with (
            tc.tile_pool(name="sb", bufs=1) as sb,
            tc.tile_pool(name="ps", bufs=2, space="PSUM") as ps,
            tc.tile_pool(name="psk", bufs=2, space="PSUM") as psk,
            tc.tile_pool(name="dram", bufs=2, space="DRAM") as dram,
        ):
            # ---------- persistent setup ----------
            sw_sb = sb.tile([128, 32], F32, tag="sw")
            nc.gpsimd.dma_start(sw_sb[:], sw_d[:])
            sws3 = sb.tile([3, PTS], F16, tag="sws3")
            nc.gpsimd.dma_start(sws3[:], sws3_d[:])
            db3_sb = sb.tile([3, NITER], F32, tag="db3")
            nc.gpsimd.dma_start(db3_sb[:], db3_d[:])

            x_cur = sb.tile([3, PTS], F32, tag="xA")
            nc.gpsimd.dma_start(x_cur[:], x0t_d[:])

            eps_sb = sb.tile([128, 1], F32, tag="epsc")
            nc.vector.memset(eps_sb[:], float(EPS))
            licol = sb.tile([128, NITER], F32, tag="licol")
            nc.vector.memset(licol[:], 0.0)
            a3col = sb.tile([3, NITER], F32, tag="a3col")
            nc.vector.memset(a3col[:], 0.0)

            # zt activation pools (3D tiles [128, NPT, PT])
            zA = [sb.tile([128, NPT, PT], F16, tag=f"zA{co}", name=f"zA{co}")
                  for co in range(8)]
            zB = [sb.tile([128, NPT, PT], F16, tag=f"zB{co}", name=f"zB{co}")
                  for co in range(4)]
            junk16 = sb.tile([128, NPT, PT], F16, tag="junk16")
            scr3 = sb.tile([3, PTS], F16, tag="scr3")
            # aliases: fp16 x for layer0 lives in junk16's first rows (safe:
            # the first junk write of an iter comes after L0's matmuls);
            # tanh preds live in zB[0]'s first rows (zB is free at L7 time).
            xf16 = junk16
            preds16 = zB[0]

            # per-iteration state (rotating buffers across iterations)
            state = {}

            def it_state(it):
                if it in state:
                    return state[it]
                s = {
                    'Ld': sb.tile([11, PTS], F16, tag="Ld", bufs=2,
                                  name=f"Ld_{it}"),
                    'Le': sb.tile([11, PTS], F16, tag="Le", bufs=1,
                                  name=f"Le_{it}"),
                    'r': [sb.tile([13, RP], F16, tag=f"r{b}", bufs=2,
                                  name=f"r_{it}_{b}") for b in range(BL)],
                    'm': sb.tile([128, 32], F32, tag="m_all", bufs=2,
                                 name=f"m_{it}"),
                    'S': sb.tile([128, 32], F32, tag="S", bufs=2,
                                 name=f"S_{it}"),
                }
                state[it] = s
                return s

            def emit_p1(it, t):
                """KNN phase1 tile: d-matmuls + row max for (b, qt)."""
                s = it_state(it)
                b, qt = t // 8, t % 8
                col = t
                qsl = slice(b * N + qt * QT, b * N + (qt + 1) * QT)
                kpd = psk.tile([QT, 2, 512], F32, tag="kpd")
                for rt in range(2):
                    nc.tensor.matmul(kpd[:, rt, :], s['Ld'][0:11, qsl],
                                     s['r'][b][0:11, rt * 512:(rt + 1) * 512],
                                     start=True, stop=True)
                nc.vector.tensor_reduce(s['m'][0:QT, col:col + 1], kpd[:],
                                        AX.XY, OP.max)

            def emit_p4(it, t):
                """KNN phase4 tile: recompute d, compute e, extract."""
                s = it_state(it)
                b, qt = t // 8, t % 8
                col = t
                qsl = slice(b * N + qt * QT, b * N + (qt + 1) * QT)
                kpd = psk.tile([QT, 2, 512], F32, tag="kpd")
                for rt in range(2):
                    nc.tensor.matmul(kpd[:, rt, :], s['Ld'][0:11, qsl],
                                     s['r'][b][0:11, rt * 512:(rt + 1) * 512],
                                     start=True, stop=True)
                kpe = psk.tile([QT, 2, 512], F32, tag="kpd")
                for rt in range(2):
                    nc.tensor.matmul(kpe[:, rt, :], s['Le'][0:11, qsl],
                                     s['r'][b][0:11, rt * 512:(rt + 1) * 512],
                                     start=True, stop=True)
                o16 = sb.tile([QT, 2, 512], F16, tag="o16", bufs=2)
                nc.vector.tensor_scalar(o16[:], kpd[:],
                                        s['m'][0:QT, col:col + 1], None,
                                        OP.is_equal)
                nc.vector.scalar_tensor_tensor(
                    o16[:], kpe[:], 1.0, o16[:], OP.mult, OP.mult,
                    accum_out=s['S'][0:QT, col:col + 1])

            # ---------- main loop ----------
            for it in range(niter):
                s = it_state(it)

                # prefetch: weights, gh, r tiles for this iteration
                wts = []
                for l in range(nlayers):
                    cin, cout = LAYERS[l][:2]
                    wt = sb.tile(
                        [cin, cout] if cin <= 128 else [128, NCI[l] * cout],
                        F16, tag=f"w{l}", name=f"w_{it}_{l}")
                    nc.gpsimd.dma_start(wt[:], w_d[it][l][:])
                    wts.append(wt)
                gh = sb.tile([128, 2, GHTOT], F32, tag="gh", bufs=2)
                nc.gpsimd.dma_start(gh[:], gh_d[it][:])
                if do_knn:
                    for b in range(BL):
                        nc.gpsimd.dma_start(s['r'][b][:], r_d[it][b][:])
                    nc.vector.memset(s['S'][:], 0.0)

                # fp16 copy of x for layer 0 (into junk16 alias rows) --
                # first, so L0's matmuls are not gated on the Ld/Le builds
                nc.vector.tensor_copy(xf16[0:3, :, :], x_cur[:])

                p1c = [0]
                p4c = [0]

                if do_knn:
                    # build Ld(it) = [2x_h, 2x_l, 2x_h, -1, -1]
                    nc.vector.tensor_scalar_mul(s['Ld'][0:3, :], x_cur[:], 2.0)
                    nc.vector.scalar_tensor_tensor(
                        scr3[:], x_cur[:], 2.0, s['Ld'][0:3, :], OP.mult,
                        OP.subtract)
                    nc.gpsimd.dma_start(s['Ld'][3:6, :], scr3[:])
                    nc.gpsimd.dma_start(s['Ld'][6:9, :], s['Ld'][0:3, :])
                    nc.gpsimd.dma_start(s['Ld'][9:11, :], cneg_d[:])

                    if it > 0:
                        # build Le(it-1) from x_cur (post-update coords of
                        # iter it-1), plus term2 for iter it-1
                        sp = it_state(it - 1)
                        nc.vector.tensor_scalar_mul(sp['Le'][0:3, :],
                                                    x_cur[:], -2.0)
                        nc.vector.scalar_tensor_tensor(
                            scr3[:], x_cur[:], -2.0, sp['Le'][0:3, :],
                            OP.mult, OP.subtract)
                        nc.gpsimd.dma_start(sp['Le'][3:6, :], scr3[:])
                        nc.gpsimd.dma_start(sp['Le'][6:9, :], sp['Le'][0:3, :])
                        nc.gpsimd.dma_start(sp['Le'][9:11, :], cpos_d[:])
                        # term2: sum_q sw_q |x|^2 via (x*sqrt(sw))^2
                        nc.vector.tensor_tensor(out=scr3[:], in0=x_cur[:],
                                                in1=sws3[:], op=OP.mult)
                        nc.vector.scalar_tensor_tensor(
                            preds16[0:3, :, :], scr3[:], 1.0, scr3[:],
                            OP.mult, OP.mult,
                            accum_out=a3col[0:3, it - 1:it])

                def gate_col(m_tile, col, gate_ap):
                    # value-preserving write to m[col] that reads gate_ap:
                    # m[col] = 0*gate + m[col].  Pins the KNN batch to the
                    # boundary (scheduler cannot hoist it before the gate).
                    nc.vector.scalar_tensor_tensor(
                        m_tile[0:QT, col:col + 1], gate_ap, 0.0,
                        m_tile[0:QT, col:col + 1], OP.mult, OP.add)

                def knn_batch(bi, gate_ap=None):
                    """Emit the KNN fill slice for boundary bi of iter it."""
                    if not do_knn:
                        return
                    p1s = [8, 8, 8, 8, 0, 0, 0] if it == 0 else P1_SPLIT
                    if gate_ap is not None and p1c[0] < 32:
                        gate_col(s['m'], p1c[0], gate_ap)
                    for _ in range(p1s[bi]):
                        if p1c[0] < 32:
                            emit_p1(it, p1c[0])
                            p1c[0] += 1
                    if it > 0:
                        if gate_ap is not None and p4c[0] < 32:
                            gate_col(it_state(it - 1)['m'], p4c[0], gate_ap)
                        for _ in range(P4_SPLIT[bi]):
                            if p4c[0] < 32:
                                emit_p4(it - 1, p4c[0])
                                p4c[0] += 1

                # ---------- MLP ----------
                zin = None
                bi = 0
                for l in range(nlayers):
                    cin, cout, has_bn = LAYERS[l]
                    nci, nco = NCI[l], NCO[l]
                    CIP = min(128, cin)
                    zout = zA if ZPOOL[l] == 'A' else zB
                    wt = wts[l]

                    if has_bn:
                        sums = sb.tile([128, nco, NG], F32, tag="sums", bufs=2)
                        ssq = sb.tile([128, nco, NG], F32, tag="ssq", bufs=2)
                        statsr = sb.tile([128, 2, nco], F32, tag="statsr",
                                         bufs=2)

                    for g in range(NG):
                        for co in range(nco):
                            CO = min(128, cout - co * 128)
                            zp = ps.tile([128, 2, 512], F32, tag="zp")
                            for ci in range(nci):
                                if l == 0:
                                    lhs = wt[0:CIP, co * 128:co * 128 + CO]
                                else:
                                    lhs = wt[0:CIP,
                                             ci * cout + co * 128:
                                             ci * cout + co * 128 + CO]
                                for h in range(2):
                                    pt = 2 * g + h
                                    if l == 0:
                                        rhs = xf16[0:3, pt, :]
                                    else:
                                        rhs = zin[ci][0:CIP, pt, :]
                                    nc.tensor.matmul(
                                        zp[0:CO, h, 0:PT], lhs, rhs,
                                        start=(ci == 0), stop=(ci == nci - 1))
                            if has_bn:
                                nc.vector.tensor_scalar(
                                    zout[co][0:CO, 2 * g:2 * g + 2, :],
                                    zp[0:CO, :, 0:PT], 1.0, 0.0, OP.mult,
                                    OP.add,
                                    accum_out=sums[0:CO, co, g:g + 1])
                                # sumsq for this (co, g) slice
                                if co % 2 == 0:
                                    nc.scalar.activation(
                                        junk16[0:CO, 2 * g:2 * g + 2, :],
                                        zout[co][0:CO, 2 * g:2 * g + 2, :],
                                        AF.Square,
                                        accum_out=ssq[0:CO, co, g:g + 1])
                                else:
                                    nc.vector.scalar_tensor_tensor(
                                        junk16[0:CO, 2 * g:2 * g + 2, :],
                                        zout[co][0:CO, 2 * g:2 * g + 2, :],
                                        1.0,
                                        zout[co][0:CO, 2 * g:2 * g + 2, :],
                                        OP.mult, OP.mult,
                                        accum_out=ssq[0:CO, co, g:g + 1])
                            else:
                                nc.scalar.activation(
                                    preds16[0:3, 2 * g:2 * g + 2, :],
                                    zp[0:3, :, 0:PT], AF.Tanh,
                                    bias=db3_sb[:, it:it + 1])

                    if not has_bn:
                        break

                    nc.vector.tensor_reduce(statsr[:, 0, :], sums[:], AX.X,
                                            OP.add)
                    nc.vector.tensor_reduce(statsr[:, 1, :], ssq[:], AX.X,
                                            OP.add)
                    arin = dram.tile([128, 2, nco], F32, tag="arin")
                    arout = dram.tile([128, 2, nco], F32, tag="arout")
                    nc.gpsimd.dma_start(arin[:], statsr[:])
                    if do_ar:
                        nc.gpsimd.collective_compute(
                            "AllReduce", OP.add, replica_groups=RG,
                            ins=[arin.opt()], outs=[arout.opt()])
                    else:
                        nc.gpsimd.dma_start(arout[:], arin[:])

                    statsg = sb.tile([128, 2, nco], F32, tag="statsg", bufs=2)
                    nc.gpsimd.dma_start(statsg[:], arout[:])

                    # affine coefs: a = g/sqrt(var+eps); ct = h - mean*a
                    af = sb.tile([128, 6, nco], F32, tag="af", bufs=2)
                    inv_n = 1.0 / NPTS_GLOBAL
                    nc.vector.tensor_scalar_mul(af[:, 0:2, :], statsg[:],
                                                inv_n)
                    nc.vector.tensor_tensor(out=af[:, 2, :], in0=af[:, 0, :],
                                            in1=af[:, 0, :], op=OP.mult)
                    nc.vector.tensor_tensor(out=af[:, 1, :], in0=af[:, 1, :],
                                            in1=af[:, 2, :], op=OP.subtract)
                    nc.scalar.activation(af[:, 2, :], af[:, 1, :], AF.Sqrt,
                                         bias=eps_sb[:])
                    nc.vector.reciprocal(af[:, 3, :], af[:, 2, :])
                    nc.vector.tensor_tensor(
                        out=af[:, 4, :], in0=gh[:, 0, GHOFF[l]:GHOFF[l] + nco],
                        in1=af[:, 3, :], op=OP.mult)
                    nc.vector.tensor_tensor(out=af[:, 2, :], in0=af[:, 0, :],
                                            in1=af[:, 4, :], op=OP.mult)
                    nc.vector.tensor_tensor(
                        out=af[:, 5, :], in0=gh[:, 1, GHOFF[l]:GHOFF[l] + nco],
                        in1=af[:, 2, :], op=OP.subtract)

                    # affine+relu at (chunk, group) granularity, g-major
                    # so the next layer's first psum chains unblock early;
                    # chunks alternate scalar/vector engines
                    for g in range(NG):
                        for co in range(nco):
                            CO = min(128, cout - co * 128)
                            dst = zout[co][0:CO, 2 * g:2 * g + 2, :]
                            sc = af[0:CO, 4, co:co + 1]
                            bi_ = af[0:CO, 5, co:co + 1]
                            if co % 2 == 0:
                                nc.scalar.activation(dst, dst, AF.Relu,
                                                     bias=bi_, scale=sc)
                            else:
                                nc.vector.tensor_scalar(
                                    dst, dst, sc, bi_, OP.mult, OP.add)
                                nc.vector.tensor_scalar_max(dst, dst, 0.0)
                    # KNN fill for the AllReduce window, emitted after the
                    # coef/affine chain so those win vector-engine priority;
                    # the stats gate still pins the fill to this boundary.
                    knn_batch(bi, statsr[0:QT, 0, 0:1])
                    bi += 1
                    zin = zout

                # ---------- x update (in place) ----------
                if nlayers < 8:
                    continue
                nc.vector.tensor_tensor(out=x_cur[:], in0=x_cur[:],
                                        in1=preds16[0:3, :, :], op=OP.add)

                if do_knn and it == niter - 1:
                    # tail: build Le(last)+term2(last), then all of phase4
                    nc.vector.tensor_scalar_mul(s['Le'][0:3, :], x_cur[:],
                                                -2.0)
                    nc.vector.scalar_tensor_tensor(
                        scr3[:], x_cur[:], -2.0, s['Le'][0:3, :], OP.mult,
                        OP.subtract)
                    nc.gpsimd.dma_start(s['Le'][3:6, :], scr3[:])
                    nc.gpsimd.dma_start(s['Le'][6:9, :], s['Le'][0:3, :])
                    nc.gpsimd.dma_start(s['Le'][9:11, :], cpos_d[:])
                    nc.vector.tensor_tensor(out=scr3[:], in0=x_cur[:],
                                            in1=sws3[:], op=OP.mult)
                    nc.vector.scalar_tensor_tensor(
                        preds16[0:3, :, :], scr3[:], 1.0, scr3[:], OP.mult,
                        OP.mult, accum_out=a3col[0:3, it:it + 1])
                    for t in range(32):
                        emit_p4(it, t)

                # loss column for iter it-1 (S complete after its 32 p4 tiles)
                if do_knn and it > 0:
                    sp = it_state(it - 1)
                    jk2 = sb.tile([128, 32], F32, tag="jk2", bufs=2)
                    nc.vector.scalar_tensor_tensor(
                        jk2[:], sp['S'][:], 1.0, sw_sb[:], OP.mult, OP.mult,
                        accum_out=licol[:, it - 1:it])

            if do_knn and niter >= 1 and nlayers >= 8:
                sl = it_state(niter - 1)
                jk2 = sb.tile([128, 32], F32, tag="jk2", bufs=2)
                nc.vector.scalar_tensor_tensor(
                    jk2[:], sl['S'][:], 1.0, sw_sb[:], OP.mult, OP.mult,
                    accum_out=licol[:, niter - 1:niter])

            lacc = sb.tile([128, 1], F32, tag="laccA")
            nc.vector.tensor_reduce(lacc[:], licol[:], AX.X, OP.add)
            acc3f = sb.tile([3, 1], F32, tag="acc3A")
            nc.vector.tensor_reduce(acc3f[:], a3col[0:3, :], AX.X, OP.add)
            nc.gpsimd.dma_start(loss_d[:], lacc[:])
            nc.gpsimd.dma_start(loss3_d[:], acc3f[:])
    nc.compile()
    return nc


def _host_prep(inputs):
    """Build per-core input maps."""
    f32 = np.float32
    noisy = np.asarray(inputs["pcl_noisy"], f32)
    clean = np.asarray(inputs["pcl_clean"], f32)
    seeds = np.asarray(inputs["pcl_seeds"], f32)
    std = np.asarray(inputs["pcl_std"], f32)
    noise = np.asarray(inputs["noise"], f32)

    pn = noisy - seeds
    pc = clean - seeds
    sdist = np.sum(pn.astype(np.float64) ** 2, -1, keepdims=True)
    max_sq = sdist[:, -1:, :]
    sw = np.exp(-sdist * 9.0 / max_sq)[..., 0]
    sw = (sw / sw.sum(1, keepdims=True))  # [B, N] float64

    tgts = []
    cur = std.copy()
    for i in range(NITER):
        if i < NITER - 1:
            cur = cur / NOISE_DECAY
            tgts.append(pc + noise[i] * cur[:, None, None])
        else:
            tgts.append(pc.copy())

    sent = np.full((RP - N, 3), 100.0, np.float64)

    shared = {}
    for i in range(NITER):
        for l in range(8):
            key = f'ew{l+1}' if l < 5 else f'dw{l-4}'
            w = np.asarray(inputs[key], f32)[i].astype(np.float16)
            cin, cout = LAYERS[l][:2]
            if cin <= 128:
                shared[f"w_{i}_{l}"] = w
            else:
                nci = NCI[l]
                shared[f"w_{i}_{l}"] = np.ascontiguousarray(
                    w.reshape(nci, 128, cout).transpose(1, 0, 2).reshape(
                        128, nci * cout))
        gharr = np.zeros((128, 2, GHTOT), f32)
        for l in range(7):
            nco = NCO[l]
            cout = LAYERS[l][1]
            gk = f'eg{l+1}' if l < 5 else f'dg{l-4}'
            hk = f'eh{l+1}' if l < 5 else f'dh{l-4}'
            g = np.asarray(inputs[gk], f32)[i]
            h = np.asarray(inputs[hk], f32)[i]
            gp = np.zeros(nco * 128, f32); gp[:cout] = g
            hp = np.zeros(nco * 128, f32); hp[:cout] = h
            gharr[:, 0, GHOFF[l]:GHOFF[l] + nco] = gp.reshape(nco, 128).T
            gharr[:, 1, GHOFF[l]:GHOFF[l] + nco] = hp.reshape(nco, 128).T
        shared[f"gh_{i}"] = gharr
    shared["db3t"] = np.ascontiguousarray(np.asarray(inputs["db3"], f32).T)
    shared["cneg"] = np.full((2, PTS), -1.0, np.float16)
    shared["cpos"] = np.full((2, PTS), 1.0, np.float16)

    in_maps = []
    for c in range(NCORES):
        bs = slice(c * BL, (c + 1) * BL)
        m = dict(shared)
        m["x0t"] = np.ascontiguousarray(
            pn[bs].transpose(2, 0, 1).reshape(3, PTS))
        swc = np.zeros((128, 32), f32)
        for b in range(BL):
            for qt in range(8):
                swc[0:QT, b * 8 + qt] = sw[c * BL + b,
                                           qt * QT:(qt + 1) * QT].astype(f32)
        m["sw"] = swc
        m["sws3"] = np.broadcast_to(
            np.sqrt(sw[bs]).reshape(1, PTS), (3, PTS)).astype(np.float16)
        for i in range(NITER):
            for b in range(BL):
                coords = np.concatenate(
                    [tgts[i][c * BL + b].astype(np.float64), sent], 0)
                rh = coords.astype(np.float16)
                rl = (coords - rh.astype(np.float64)).astype(np.float16)
                rsq = (coords ** 2).sum(1)
                rsqh = rsq.astype(np.float16)
                rsql = (rsq - rsqh.astype(np.float64)).astype(np.float16)
                R = np.empty((13, RP), np.float16)
                R[0:3] = rh.T
                R[3:6] = rh.T
                R[6:9] = rl.T
                R[9] = rsqh
                R[10] = rsql
                R[11] = 1.0
                R[12] = 1.0
                m[f"rknn_{i}_{b}"] = R
        in_maps.append(m)
    return in_maps


def kernel(**inputs):
    if "nc" not in _NC_CACHE:
        _NC_CACHE["nc"] = _build()
    nc = _NC_CACHE["nc"]
    in_maps = _host_prep(inputs)
    res = run_bass_kernel_spmd(nc, in_maps, list(range(NCORES))).results
    total = 0.0
    for c in range(NCORES):
        total += float(res[c]["loss_part"].sum())
        total += float(res[c]["loss_part3"].sum())
    return np.asarray(total / B, dtype=np.float32)


# revision 22
# speedup vs baseline: 1.0442x; 1.0014x over previous
"""DenoiseNet loss kernel for 8 Trainium2 NeuronCores.

Strategy: pure data parallel over the batch (4 batches/core). PointNet MLP in
fp16 (fp32 PSUM accumulate), exact global BatchNorm via per-layer AllReduce of
(sum, sumsq). KNN argmin via hi/lo-fp16-split matmul of 2q.r-|r|^2, extraction
of the matched |v-r*|^2 by is_equal one-hot + fused multiply-reduce.

v2 restructure for PE warmth / engine balance:
 - weights host-packed per layer, one DMA per (iter, layer), prefetched at
   iteration start.
 - PSUM evictions on the vector engine with fused sum-accumulate; sumsq
   pass split between scalar/vector; BN affine+ReLU fused into a single
   scalar-engine activation per 128-channel chunk (true per-channel scale,
   no weight folding).
 - KNN matmuls of phase1(iter i) and phase4(iter i-1) are interleaved into
   the 7 BN-AllReduce stall windows of iter i, keeping the PE busy (HAM
   stays at full clock) while the collective is in flight.
"""
import numpy as np

import concourse.bass as bass
import concourse.mybir as mybir
import concourse.tile as tile
from concourse import bacc
from concourse.bass_utils import run_bass_kernel_spmd

dt = mybir.dt
F32 = dt.float32
F16 = dt.float16
AF = mybir.ActivationFunctionType
OP = mybir.AluOpType
AX = mybir.AxisListType

B, N, NCORES = 32, 1000, 8
BL = B // NCORES            # 4 batches per core
PTS = BL * N                # 4000 points per core
NITER = 4
NPTS_GLOBAL = B * N         # 32000 (BN population)
EPS = 1e-5
NOISE_DECAY = 4.0
QT = 125                    # q tile (8 per batch)
RP = 1024                   # padded ref points (24 sentinels)
PT = 500                    # pts tile (8 tiles of 500)
NPT = PTS // PT             # 8
NG = NPT // 2               # 4 psum groups of 2 tiles

# (C_in, C_out, has_bn) per layer
LAYERS = [(3, 64, 1), (64, 128, 1), (128, 256, 1), (256, 512, 1),
          (512, 1024, 1), (1024, 512, 1), (512, 256, 1), (256, 3, 0)]
NCI = [max(1, ci // 128) for ci, co, _ in LAYERS]
NCO = [max(1, (co + 127) // 128) for ci, co, _ in LAYERS]
# gh packing offsets for the 7 BN layers
GHOFF = [0]
for _l in range(7):
    GHOFF.append(GHOFF[-1] + NCO[_l])
GHTOT = GHOFF[-1]           # 22

# zt buffer assignment: even layers write pool A (8 bufs), odd write pool B (4)
ZPOOL = ['A', 'B', 'A', 'B', 'A', 'B', 'A', None]

# KNN fill distribution: tiles emitted at each of the 7 BN boundaries
P1_SPLIT = [5, 5, 5, 5, 4, 4, 4]
P4_SPLIT = [5, 5, 4, 4, 4, 4, 4]

RG = [list(range(NCORES))]

_NC_CACHE = {}


def _build(niter=NITER, nlayers=8, do_knn=True, do_ar=True):
    nc = bacc.Bacc(None, target_bir_lowering=False, debug=False)

    x0t_d = nc.dram_tensor("x0t", [3, PTS], F32, kind="ExternalInput")
    cneg_d = nc.dram_tensor("cneg", [2, PTS], F16, kind="ExternalInput")
    cpos_d = nc.dram_tensor("cpos", [2, PTS], F16, kind="ExternalInput")
    sw_d = nc.dram_tensor("sw", [128, 32], F32, kind="ExternalInput")
    sws3_d = nc.dram_tensor("sws3", [3, PTS], F16, kind="ExternalInput")
    db3_d = nc.dram_tensor("db3t", [3, NITER], F32, kind="ExternalInput")
    r_d = [[nc.dram_tensor(f"rknn_{i}_{b}", [13, RP], F16, kind="ExternalInput")
            for b in range(BL)] for i in range(NITER)]
    w_d = [[nc.dram_tensor(
        f"w_{i}_{l}",
        [LAYERS[l][0], LAYERS[l][1]] if LAYERS[l][0] <= 128
        else [128, NCI[l] * LAYERS[l][1]], F16, kind="ExternalInput")
        for l in range(8)] for i in range(NITER)]
    gh_d = [nc.dram_tensor(f"gh_{i}", [128, 2, GHTOT], F32,
                           kind="ExternalInput") for i in range(NITER)]
    loss_d = nc.dram_tensor("loss_part", [128, 1], F32, kind="ExternalOutput")
    loss3_d = nc.dram_tensor("loss_part3", [3, 1], F32, kind="ExternalOutput")

    with tile.TileContext(nc) as tc:
        with (
            tc.tile_pool(name="sb", bufs=1) as sb,
            tc.tile_pool(name="ps", bufs=2, space="PSUM") as ps,
            tc.tile_pool(name="psk", bufs=2, space="PSUM") as psk,
            tc.tile_pool(name="dram", bufs=2, space="DRAM") as dram,
        ):
            # ---------- persistent setup ----------
            sw_sb = sb.tile([128, 32], F32, tag="sw")
            nc.gpsimd.dma_start(sw_sb[:], sw_d[:])
            sws3 = sb.tile([3, PTS], F16, tag="sws3")
            nc.gpsimd.dma_start(sws3[:], sws3_d[:])
            db3_sb = sb.tile([3, NITER], F32, tag="db3")
            nc.gpsimd.dma_start(db3_sb[:], db3_d[:])

            x_cur = sb.tile([3, PTS], F32, tag="xA")
            nc.gpsimd.dma_start(x_cur[:], x0t_d[:])

            eps_sb = sb.tile([128, 1], F32, tag="epsc")
            nc.vector.memset(eps_sb[:], float(EPS))
            licol = sb.tile([128, NITER], F32, tag="licol")
            nc.vector.memset(licol[:], 0.0)
            a3col = sb.tile([3, NITER], F32, tag="a3col")
            nc.vector.memset(a3col[:], 0.0)

            # zt activation pools (3D tiles [128, NPT, PT])
            zA = [sb.tile([128, NPT, PT], F16, tag=f"zA{co}", name=f"zA{co}")
                  for co in range(8)]
            zB = [sb.tile([128, NPT, PT], F16, tag=f"zB{co}", name=f"zB{co}")
                  for co in range(4)]
            junk16 = sb.tile([128, NPT, PT], F16, tag="junk16")
            scr3 = sb.tile([3, PTS], F16, tag="scr3")
            # aliases: fp16 x for layer0 lives in junk16's first rows (safe:
            # the first junk write of an iter comes after L0's matmuls);
            # tanh preds live in zB[0]'s first rows (zB is free at L7 time).
            xf16 = junk16
            preds16 = zB[0]

            # per-iteration state (rotating buffers across iterations)
            state = {}

            def it_state(it):
                if it in state:
                    return state[it]
                s = {
                    'Ld': sb.tile([11, PTS], F16, tag="Ld", bufs=2,
                                  name=f"Ld_{it}"),
                    'Le': sb.tile([11, PTS], F16, tag="Le", bufs=1,
                                  name=f"Le_{it}"),
                    'r': [sb.tile([13, RP], F16, tag=f"r{b}", bufs=2,
                                  name=f"r_{it}_{b}") for b in range(BL)],
                    'm': sb.tile([128, 32], F32, tag="m_all", bufs=2,
                                 name=f"m_{it}"),
                    'S': sb.tile([128, 32], F32, tag="S", bufs=2,
                                 name=f"S_{it}"),
                }
                state[it] = s
                return s

            def emit_p1(it, t):
                """KNN phase1 tile: d-matmuls + row max for (b, qt)."""
                s = it_state(it)
                b, qt = t // 8, t % 8
                col = t
                qsl = slice(b * N + qt * QT, b * N + (qt + 1) * QT)
                kpd = psk.tile([QT, 2, 512], F32, tag="kpd")
                for rt in range(2):
                    nc.tensor.matmul(kpd[:, rt, :], s['Ld'][0:11, qsl],
                                     s['r'][b][0:11, rt * 512:(rt + 1) * 512],
                                     start=True, stop=True)
                nc.vector.tensor_reduce(s['m'][0:QT, col:col + 1], kpd[:],
                                        AX.XY, OP.max)

            def emit_p4(it, t):
                """KNN phase4 tile: recompute d, compute e, extract."""
                s = it_state(it)
                b, qt = t // 8, t % 8
                col = t
                qsl = slice(b * N + qt * QT, b * N + (qt + 1) * QT)
                kpd = psk.tile([QT, 2, 512], F32, tag="kpd")
                for rt in range(2):
                    nc.tensor.matmul(kpd[:, rt, :], s['Ld'][0:11, qsl],
                                     s['r'][b][0:11, rt * 512:(rt + 1) * 512],
                                     start=True, stop=True)
                kpe = psk.tile([QT, 2, 512], F32, tag="kpd")
                for rt in range(2):
                    nc.tensor.matmul(kpe[:, rt, :], s['Le'][0:11, qsl],
                                     s['r'][b][0:11, rt * 512:(rt + 1) * 512],
                                     start=True, stop=True)
                o16 = sb.tile([QT, 2, 512], F16, tag="o16", bufs=2)
                nc.vector.tensor_scalar(o16[:], kpd[:],
                                        s['m'][0:QT, col:col + 1], None,
                                        OP.is_equal)
                nc.vector.scalar_tensor_tensor(
                    o16[:], kpe[:], 1.0, o16[:], OP.mult, OP.mult,
                    accum_out=s['S'][0:QT, col:col + 1])

            # ---------- main loop ----------
            for it in range(niter):
                s = it_state(it)

                # prefetch: weights, gh, r tiles for this iteration
                wts = []
                for l in range(nlayers):
                    cin, cout = LAYERS[l][:2]
                    wt = sb.tile(
                        [cin, cout] if cin <= 128 else [128, NCI[l] * cout],
                        F16, tag=f"w{l}", name=f"w_{it}_{l}")
                    nc.gpsimd.dma_start(wt[:], w_d[it][l][:])
                    wts.append(wt)
                gh = sb.tile([128, 2, GHTOT], F32, tag="gh", bufs=2)
                nc.gpsimd.dma_start(gh[:], gh_d[it][:])
                if do_knn:
                    for b in range(BL):
                        nc.gpsimd.dma_start(s['r'][b][:], r_d[it][b][:])
                    nc.vector.memset(s['S'][:], 0.0)

                # fp16 copy of x for layer 0 (into junk16 alias rows) --
                # first, so L0's matmuls are not gated on the Ld/Le builds
                nc.vector.tensor_copy(xf16[0:3, :, :], x_cur[:])

                p1c = [0]
                p4c = [0]

                if do_knn:
                    # build Ld(it) = [2x_h, 2x_l, 2x_h, -1, -1]
                    nc.vector.tensor_scalar_mul(s['Ld'][0:3, :], x_cur[:], 2.0)
                    nc.vector.scalar_tensor_tensor(
                        scr3[:], x_cur[:], 2.0, s['Ld'][0:3, :], OP.mult,
                        OP.subtract)
                    nc.gpsimd.dma_start(s['Ld'][3:6, :], scr3[:])
                    nc.gpsimd.dma_start(s['Ld'][6:9, :], s['Ld'][0:3, :])
                    nc.gpsimd.dma_start(s['Ld'][9:11, :], cneg_d[:])

                    if it > 0:
                        # build Le(it-1) from x_cur (post-update coords of
                        # iter it-1), plus term2 for iter it-1
                        sp = it_state(it - 1)
                        nc.vector.tensor_scalar_mul(sp['Le'][0:3, :],
                                                    x_cur[:], -2.0)
                        nc.vector.scalar_tensor_tensor(
                            scr3[:], x_cur[:], -2.0, sp['Le'][0:3, :],
                            OP.mult, OP.subtract)
                        nc.gpsimd.dma_start(sp['Le'][3:6, :], scr3[:])
                        nc.gpsimd.dma_start(sp['Le'][6:9, :], sp['Le'][0:3, :])
                        nc.gpsimd.dma_start(sp['Le'][9:11, :], cpos_d[:])
                        # term2: sum_q sw_q |x|^2 via (x*sqrt(sw))^2
                        nc.vector.tensor_tensor(out=scr3[:], in0=x_cur[:],
                                                in1=sws3[:], op=OP.mult)
                        nc.vector.scalar_tensor_tensor(
                            preds16[0:3, :, :], scr3[:], 1.0, scr3[:],
                            OP.mult, OP.mult,
                            accum_out=a3col[0:3, it - 1:it])

                def gate_col(m_tile, col, gate_ap):
                    # value-preserving write to m[col] that reads gate_ap:
                    # m[col] = 0*gate + m[col].  Pins the KNN batch to the
                    # boundary (scheduler cannot hoist it before the gate).
                    nc.vector.scalar_tensor_tensor(
                        m_tile[0:QT, col:col + 1], gate_ap, 0.0,
                        m_tile[0:QT, col:col + 1], OP.mult, OP.add)

                def knn_batch(bi, gate_ap=None):
                    """Emit the KNN fill slice for boundary bi of iter it."""
                    if not do_knn:
                        return
                    p1s = [8, 8, 8, 8, 0, 0, 0] if it == 0 else P1_SPLIT
                    if gate_ap is not None and p1c[0] < 32:
                        gate_col(s['m'], p1c[0], gate_ap)
                    for _ in range(p1s[bi]):
                        if p1c[0] < 32:
                            emit_p1(it, p1c[0])
                            p1c[0] += 1
                    if it > 0:
                        if gate_ap is not None and p4c[0] < 32:
                            gate_col(it_state(it - 1)['m'], p4c[0], gate_ap)
                        for _ in range(P4_SPLIT[bi]):
                            if p4c[0] < 32:
                                emit_p4(it - 1, p4c[0])
                                p4c[0] += 1

                # ---------- MLP ----------
                zin = None
                bi = 0
                for l in range(nlayers):
                    cin, cout, has_bn = LAYERS[l]
                    nci, nco = NCI[l], NCO[l]
                    CIP = min(128, cin)
                    zout = zA if ZPOOL[l] == 'A' else zB
                    wt = wts[l]

                    if has_bn:
                        sums = sb.tile([128, nco, NG], F32, tag="sums", bufs=2)
                        ssq = sb.tile([128, nco, NG], F32, tag="ssq", bufs=2)
                        statsr = sb.tile([128, 2, nco], F32, tag="statsr",
                                         bufs=2)

                    for g in range(NG):
                        for co in range(nco):
                            CO = min(128, cout - co * 128)
                            zp = ps.tile([128, 2, 512], F32, tag="zp")
                            for ci in range(nci):
                                if l == 0:
                                    lhs = wt[0:CIP, co * 128:co * 128 + CO]
                                else:
                                    lhs = wt[0:CIP,
                                             ci * cout + co * 128:
                                             ci * cout + co * 128 + CO]
                                for h in range(2):
                                    pt = 2 * g + h
                                    if l == 0:
                                        rhs = xf16[0:3, pt, :]
                                    else:
                                        rhs = zin[ci][0:CIP, pt, :]
                                    nc.tensor.matmul(
                                        zp[0:CO, h, 0:PT], lhs, rhs,
                                        start=(ci == 0), stop=(ci == nci - 1))
                            if has_bn:
                                nc.vector.tensor_scalar(
                                    zout[co][0:CO, 2 * g:2 * g + 2, :],
                                    zp[0:CO, :, 0:PT], 1.0, 0.0, OP.mult,
                                    OP.add,
                                    accum_out=sums[0:CO, co, g:g + 1])
                                # sumsq for this (co, g) slice
                                if co % 2 == 0:
                                    nc.scalar.activation(
                                        junk16[0:CO, 2 * g:2 * g + 2, :],
                                        zout[co][0:CO, 2 * g:2 * g + 2, :],
                                        AF.Square,
                                        accum_out=ssq[0:CO, co, g:g + 1])
                                else:
                                    nc.vector.scalar_tensor_tensor(
                                        junk16[0:CO, 2 * g:2 * g + 2, :],
                                        zout[co][0:CO, 2 * g:2 * g + 2, :],
                                        1.0,
                                        zout[co][0:CO, 2 * g:2 * g + 2, :],
                                        OP.mult, OP.mult,
                                        accum_out=ssq[0:CO, co, g:g + 1])
                            else:
                                nc.scalar.activation(
                                    preds16[0:3, 2 * g:2 * g + 2, :],
                                    zp[0:3, :, 0:PT], AF.Tanh,
                                    bias=db3_sb[:, it:it + 1])

                    if not has_bn:
                        break

                    nc.vector.tensor_reduce(statsr[:, 0, :], sums[:], AX.X,
                                            OP.add)
                    nc.vector.tensor_reduce(statsr[:, 1, :], ssq[:], AX.X,
                                            OP.add)
                    arin = dram.tile([128, 2, nco], F32, tag="arin")
                    arout = dram.tile([128, 2, nco], F32, tag="arout")
                    nc.gpsimd.dma_start(arin[:], statsr[:])
                    if do_ar:
                        nc.gpsimd.collective_compute(
                            "AllReduce", OP.add, replica_groups=RG,
                            ins=[arin.opt()], outs=[arout.opt()])
                    else:
                        nc.gpsimd.dma_start(arout[:], arin[:])

                    statsg = sb.tile([128, 2, nco], F32, tag="statsg", bufs=2)
                    nc.gpsimd.dma_start(statsg[:], arout[:])

                    # affine coefs: a = g/sqrt(var+eps); ct = h - mean*a
                    af = sb.tile([128, 6, nco], F32, tag="af", bufs=2)
                    inv_n = 1.0 / NPTS_GLOBAL
                    nc.vector.tensor_scalar_mul(af[:, 0:2, :], statsg[:],
                                                inv_n)
                    nc.vector.tensor_tensor(out=af[:, 2, :], in0=af[:, 0, :],
                                            in1=af[:, 0, :], op=OP.mult)
                    nc.vector.tensor_tensor(out=af[:, 1, :], in0=af[:, 1, :],
                                            in1=af[:, 2, :], op=OP.subtract)
                    nc.scalar.activation(af[:, 2, :], af[:, 1, :], AF.Sqrt,
                                         bias=eps_sb[:])
                    nc.vector.reciprocal(af[:, 3, :], af[:, 2, :])
                    nc.vector.tensor_tensor(
                        out=af[:, 4, :], in0=gh[:, 0, GHOFF[l]:GHOFF[l] + nco],
                        in1=af[:, 3, :], op=OP.mult)
                    nc.vector.tensor_tensor(out=af[:, 2, :], in0=af[:, 0, :],
                                            in1=af[:, 4, :], op=OP.mult)
                    nc.vector.tensor_tensor(
                        out=af[:, 5, :], in0=gh[:, 1, GHOFF[l]:GHOFF[l] + nco],
                        in1=af[:, 2, :], op=OP.subtract)

                    # affine+relu at (chunk, group) granularity, g-major
                    # so the next layer's first psum chains unblock early;
                    # chunks alternate scalar/vector engines
                    for g in range(NG):
                        for co in range(nco):
                            CO = min(128, cout - co * 128)
                            dst = zout[co][0:CO, 2 * g:2 * g + 2, :]
                            sc = af[0:CO, 4, co:co + 1]
                            bi_ = af[0:CO, 5, co:co + 1]
                            if co % 2 == 0:
                                nc.scalar.activation(dst, dst, AF.Relu,
                                                     bias=bi_, scale=sc)
                            else:
                                nc.vector.tensor_scalar(
                                    dst, dst, sc, bi_, OP.mult, OP.add)
                                nc.vector.tensor_scalar_max(dst, dst, 0.0)
                    # KNN fill for the AllReduce window, emitted after the
                    # coef/affine chain so those win vector-engine priority;
                    # the stats gate still pins the fill to this boundary.
                    knn_batch(bi, statsr[0:QT, 0, 0:1])
                    bi += 1
                    zin = zout

                # ---------- x update (in place) ----------
                if nlayers < 8:
                    continue
                nc.vector.tensor_tensor(out=x_cur[:], in0=x_cur[:],
                                        in1=preds16[0:3, :, :], op=OP.add)

                if do_knn and it > 0:
                    # leftover phase4 tiles: fill the L7/transition stretch
                    while p4c[0] < 32:
                        emit_p4(it - 1, p4c[0])
                        p4c[0] += 1

                if do_knn and it == niter - 1:
                    # tail: build Le(last)+term2(last), then all of phase4
                    nc.vector.tensor_scalar_mul(s['Le'][0:3, :], x_cur[:],
                                                -2.0)
                    nc.vector.scalar_tensor_tensor(
                        scr3[:], x_cur[:], -2.0, s['Le'][0:3, :], OP.mult,
                        OP.subtract)
                    nc.gpsimd.dma_start(s['Le'][3:6, :], scr3[:])
                    nc.gpsimd.dma_start(s['Le'][6:9, :], s['Le'][0:3, :])
                    nc.gpsimd.dma_start(s['Le'][9:11, :], cpos_d[:])
                    nc.vector.tensor_tensor(out=scr3[:], in0=x_cur[:],
                                            in1=sws3[:], op=OP.mult)
                    nc.vector.scalar_tensor_tensor(
                        preds16[0:3, :, :], scr3[:], 1.0, scr3[:], OP.mult,
                        OP.mult, accum_out=a3col[0:3, it:it + 1])
                    for t in range(32):
                        emit_p4(it, t)

                # loss column for iter it-1 (S complete after its 32 p4 tiles)
                if do_knn and it > 0:
                    sp = it_state(it - 1)
                    jk2 = sb.tile([128, 32], F32, tag="jk2", bufs=2)
                    nc.vector.scalar_tensor_tensor(
                        jk2[:], sp['S'][:], 1.0, sw_sb[:], OP.mult, OP.mult,
                        accum_out=licol[:, it - 1:it])

            if do_knn and niter >= 1 and nlayers >= 8:
                sl = it_state(niter - 1)
                jk2 = sb.tile([128, 32], F32, tag="jk2", bufs=2)
                nc.vector.scalar_tensor_tensor(
                    jk2[:], sl['S'][:], 1.0, sw_sb[:], OP.mult, OP.mult,
                    accum_out=licol[:, niter - 1:niter])

            lacc = sb.tile([128, 1], F32, tag="laccA")
            nc.vector.tensor_reduce(lacc[:], licol[:], AX.X, OP.add)
            acc3f = sb.tile([3, 1], F32, tag="acc3A")
            nc.vector.tensor_reduce(acc3f[:], a3col[0:3, :], AX.X, OP.add)
            nc.gpsimd.dma_start(loss_d[:], lacc[:])
            nc.gpsimd.dma_start(loss3_d[:], acc3f[:])
    nc.compile()
    return nc


def _host_prep(inputs):
    """Build per-core input maps."""
    f32 = np.float32
    noisy = np.asarray(inputs["pcl_noisy"], f32)
    clean = np.asarray(inputs["pcl_clean"], f32)
    seeds = np.asarray(inputs["pcl_seeds"], f32)
    std = np.asarray(inputs["pcl_std"], f32)
    noise = np.asarray(inputs["noise"], f32)

    pn = noisy - seeds
    pc = clean - seeds
    sdist = np.sum(pn.astype(np.float64) ** 2, -1, keepdims=True)
    max_sq = sdist[:, -1:, :]
    sw = np.exp(-sdist * 9.0 / max_sq)[..., 0]
    sw = (sw / sw.sum(1, keepdims=True))  # [B, N] float64

    tgts = []
    cur = std.copy()
    for i in range(NITER):
        if i < NITER - 1:
            cur = cur / NOISE_DECAY
            tgts.append(pc + noise[i] * cur[:, None, None])
        else:
            tgts.append(pc.copy())

    sent = np.full((RP - N, 3), 100.0, np.float64)

    shared = {}
    for i in range(NITER):
        for l in range(8):
            key = f'ew{l+1}' if l < 5 else f'dw{l-4}'
            w = np.asarray(inputs[key], f32)[i].astype(np.float16)
            cin, cout = LAYERS[l][:2]
            if cin <= 128:
                shared[f"w_{i}_{l}"] = w
            else:
                nci = NCI[l]
                shared[f"w_{i}_{l}"] = np.ascontiguousarray(
                    w.reshape(nci, 128, cout).transpose(1, 0, 2).reshape(
                        128, nci * cout))
        gharr = np.zeros((128, 2, GHTOT), f32)
        for l in range(7):
            nco = NCO[l]
            cout = LAYERS[l][1]
            gk = f'eg{l+1}' if l < 5 else f'dg{l-4}'
            hk = f'eh{l+1}' if l < 5 else f'dh{l-4}'
            g = np.asarray(inputs[gk], f32)[i]
            h = np.asarray(inputs[hk], f32)[i]
            gp = np.zeros(nco * 128, f32); gp[:cout] = g
            hp = np.zeros(nco * 128, f32); hp[:cout] = h
            gharr[:, 0, GHOFF[l]:GHOFF[l] + nco] = gp.reshape(nco, 128).T
            gharr[:, 1, GHOFF[l]:GHOFF[l] + nco] = hp.reshape(nco, 128).T
        shared[f"gh_{i}"] = gharr
    shared["db3t"] = np.ascontiguousarray(np.asarray(inputs["db3"], f32).T)
    shared["cneg"] = np.full((2, PTS), -1.0, np.float16)
    shared["cpos"] = np.full((2, PTS), 1.0, np.float16)

    in_maps = []
    for c in range(NCORES):
        bs = slice(c * BL, (c + 1) * BL)
        m = dict(shared)
        m["x0t"] = np.ascontiguousarray(
            pn[bs].transpose(2, 0, 1).reshape(3, PTS))
        swc = np.zeros((128, 32), f32)
        for b in range(BL):
            for qt in range(8):
                swc[0:QT, b * 8 + qt] = sw[c * BL + b,
                                           qt * QT:(qt + 1) * QT].astype(f32)
        m["sw"] = swc
        m["sws3"] = np.broadcast_to(
            np.sqrt(sw[bs]).reshape(1, PTS), (3, PTS)).astype(np.float16)
        for i in range(NITER):
            for b in range(BL):
                coords = np.concatenate(
                    [tgts[i][c * BL + b].astype(np.float64), sent], 0)
                rh = coords.astype(np.float16)
                rl = (coords - rh.astype(np.float64)).astype(np.float16)
                rsq = (coords ** 2).sum(1)
                rsqh = rsq.astype(np.float16)
                rsql = (rsq - rsqh.astype(np.float64)).astype(np.float16)
                R = np.empty((13, RP), np.float16)
                R[0:3] = rh.T
                R[3:6] = rh.T
                R[6:9] = rl.T
                R[9] = rsqh
                R[10] = rsql
                R[11] = 1.0
                R[12] = 1.0
                m[f"rknn_{i}_{b}"] = R
        in_maps.append(m)
    return in_maps


def kernel(**inputs):
    if "nc" not in _NC_CACHE:
        _NC_CACHE["nc"] = _build()
    nc = _NC_CACHE["nc"]
    in_maps = _host_prep(inputs)
    res = run_bass_kernel_spmd(nc, in_maps, list(range(NCORES))).results
    total = 0.0
    for c in range(NCORES):
        total += float(res[c]["loss_part"].sum())
        total += float(res[c]["loss_part3"].sum())
    return np.asarray(total / B, dtype=np.float32)
